# revision 1
# baseline (speedup 1.0000x reference)
"""Trainium2 Bass kernel for nn_AttnConvLayer (GNN message passing).

Edge-parallel, dst-sharded across 8 NeuronCores:
  - Host packs fp16 gather tables (256B rows):
      t_s [N_S,128] = [m_ss(64) | qm_ss(1) | pad]
      t_o [N_O,128] = [m_os(64) | qm_os(1) | pad]
      t_x [N_O,128] = [t_in(64) | t_out(64)]
  - Edges sharded by dst (12500/core), bucketed into 128-node dst
    windows; src split in 4 quadrant bases of 25000 rows so local
    indices fit dma_gather's int16. Each (window, quadrant) has a fixed
    budget of 3x128 edge slots; rare overflow is host-corrected.
  - Device: per (quadrant, supergroup of 14 windows) one 5376-row
    dma_gather; per-edge nom = exp(leaky_relu(qm + c)); one-hot scatter
    matmuls accumulate per-window [M,128] sums in PSUM; outputs written
    feature-major.
  - Host post: fold efeat sums through W2 (tiny matvec), divide by den,
    dense x-path (relu/Wo), transpose.
"""

import sys
sys.path.insert(0, '/opt/trn_rl_repo')
import numpy as np

from concourse import bass, bacc, mybir
import concourse.tile as tile
from concourse.bass_utils import run_bass_kernel_spmd

N_S = 100000
N_O = 100000
E = 1000000
D = 64
NC = 8
SHARD = N_S // NC          # 12500
WIN = 128
NWIN = (SHARD + WIN - 1) // WIN   # 98
SG = 14                    # windows per supergroup
NSG = NWIN // SG           # 7
B = 3                      # chunks per (window, quadrant)
NQ = 4
QD = 25000
SLOTS_WQ = B * WIN                    # 384
TOK = SG * SLOTS_WQ                   # 5376
CHUNKS_SG = SG * B                    # 42
NODES = NWIN * WIN                    # 12544

F16 = mybir.dt.float16
F32 = mybir.dt.float32
I16 = mybir.dt.int16

TYPES = [
    ("ss", 10, True),
    ("os", 2, True),
    ("fw", 0, False),
    ("bw", 0, False),
]

_PROGRAM = None


def _build_program():
    nc = bacc.Bacc(None, target_bir_lowering=False, dynamic_dma_scratch_size=2 ** 15)

    inp = {}
    inp["t_s"] = nc.declare_dram_parameter("t_s", [N_S, 128], F16, isOutput=False)
    inp["t_o"] = nc.declare_dram_parameter("t_o", [N_O, 128], F16, isOutput=False)
    inp["t_x"] = nc.declare_dram_parameter("t_x", [N_O, 128], F16, isOutput=False)
    for t, ext, attn in TYPES:
        inp[f"idx_{t}"] = nc.declare_dram_parameter(
            f"idx_{t}", [NQ, NSG, 128, TOK // 16], I16, isOutput=False)
        inp[f"dr_{t}"] = nc.declare_dram_parameter(
            f"dr_{t}", [NQ, NSG, 128, CHUNKS_SG, 1], F16, isOutput=False)
        if attn:
            inp[f"c_{t}"] = nc.declare_dram_parameter(
                f"c_{t}", [NQ, NSG, 128, CHUNKS_SG, 1], F32, isOutput=False)
            inp[f"ef_{t}"] = nc.declare_dram_parameter(
                f"ef_{t}", [NQ, NSG, 128, CHUNKS_SG, ext], F16, isOutput=False)
    inp["iota"] = nc.declare_dram_parameter("iota", [128, 1, 128], F16, isOutput=False)

    out_ss = nc.declare_dram_parameter("A_ss", [75, NODES], F32, isOutput=True)
    out_os = nc.declare_dram_parameter("A_os", [67, NODES], F32, isOutput=True)
    out_fw = nc.declare_dram_parameter("A_fw", [64, NODES], F32, isOutput=True)
    out_bw = nc.declare_dram_parameter("A_bw", [64, NODES], F32, isOutput=True)
    outs = {"ss": out_ss, "os": out_os, "fw": out_fw, "bw": out_bw}
    tables = {"ss": inp["t_s"], "os": inp["t_o"], "fw": inp["t_x"], "bw": inp["t_x"]}
    vcol = {"ss": (0, 64), "os": (0, 64), "fw": (0, 64), "bw": (64, 128)}

    with tile.TileContext(nc) as tc:
        with (
            tc.tile_pool(name="const", bufs=1) as cpool,
            tc.tile_pool(name="work", bufs=1) as pool,
            tc.tile_pool(name="stage", bufs=3) as spool,
            tc.tile_pool(name="psum", bufs=8, space="PSUM") as pp,
        ):
            iota_t = cpool.tile([128, 1, 128], F16)
            nc.sync.dma_start(out=iota_t[:, :, :], in_=inp["iota"][:, :, :])

            for t, ext, attn in TYPES:
                M = 64 + ext + (1 if attn else 0)
                table = tables[t]
                c0, c1 = vcol[t]
                for sg in range(NSG):
                    lands = []
                    Us = []
                    Ss = []
                    for q in range(NQ):
                        idx_t = pool.tile([128, TOK // 16], I16, tag=f"idx{q}")
                        nc.sync.dma_start(out=idx_t[:, :], in_=inp[f"idx_{t}"][q, sg, :, :])
                        land = pool.tile([128, CHUNKS_SG, 128], F16, tag=f"land{q}")
                        nc.gpsimd.dma_gather(
                            out_ap=land[:, :, :],
                            in_ap=table[q * QD:(q + 1) * QD, :],
                            idxs_ap=idx_t[:, :],
                            num_idxs=TOK,
                            num_idxs_reg=TOK,
                            elem_size=128,
                            single_packet=False,
                        )
                        lands.append(land)

                        dr_t = pool.tile([128, CHUNKS_SG, 1], F16, tag=f"dr{q}")
                        nc.sync.dma_start(out=dr_t[:, :, :], in_=inp[f"dr_{t}"][q, sg, :, :, :])
                        S = pool.tile([128, CHUNKS_SG, 128], F16, tag=f"S{q}")
                        nc.vector.tensor_tensor(
                            out=S[:, :, :],
                            in0=dr_t[:, :, :].to_broadcast([128, CHUNKS_SG, 128]),
                            in1=iota_t[:, :, :].to_broadcast([128, CHUNKS_SG, 128]),
                            op=mybir.AluOpType.is_equal,
                        )
                        Ss.append(S)

                        if attn:
                            cc_t = pool.tile([128, CHUNKS_SG, 1], F32, tag=f"cc{q}")
                            nc.sync.dma_start(out=cc_t[:, :, :], in_=inp[f"c_{t}"][q, sg, :, :, :])
                            ef_t = pool.tile([128, CHUNKS_SG, ext], F16, tag=f"ef{q}")
                            nc.sync.dma_start(out=ef_t[:, :, :], in_=inp[f"ef_{t}"][q, sg, :, :, :])
                            # s = qm + c (f32), lrelu, nom = exp -> fp16
                            qmf = pool.tile([128, CHUNKS_SG, 1], F32, tag=f"qmf{q}")
                            nc.vector.tensor_copy(out=qmf[:, :, :], in_=land[:, :, 64:65])
                            sv = pool.tile([128, CHUNKS_SG, 1], F32, tag=f"sv{q}")
                            nc.vector.tensor_tensor(
                                out=sv[:, :, :], in0=qmf[:, :, :], in1=cc_t[:, :, :],
                                op=mybir.AluOpType.add)
                            nc.scalar.activation(
                                sv[:, :, :], sv[:, :, :],
                                mybir.ActivationFunctionType.Lrelu, alpha=0.01)
                            nom = pool.tile([128, CHUNKS_SG, 1], F16, tag=f"nom{q}")
                            nc.scalar.activation(
                                nom[:, :, :], sv[:, :, :],
                                mybir.ActivationFunctionType.Exp)
                            U = pool.tile([128, CHUNKS_SG, M], F16, tag=f"U{q}")
                            nc.vector.tensor_tensor(
                                out=U[:, :, 0:64], in0=land[:, :, 0:64],
                                in1=nom[:, :, :].to_broadcast([128, CHUNKS_SG, 64]),
                                op=mybir.AluOpType.mult)
                            nc.vector.tensor_tensor(
                                out=U[:, :, 64:64 + ext], in0=ef_t[:, :, :],
                                in1=nom[:, :, :].to_broadcast([128, CHUNKS_SG, ext]),
                                op=mybir.AluOpType.mult)
                            nc.vector.tensor_copy(
                                out=U[:, :, M - 1:M], in_=nom[:, :, :])
                            Us.append(U)
                        else:
                            Us.append(None)

                    stage = spool.tile([M, SG * 128], F32, tag="stage")
                    for wl in range(SG):
                        ps = pp.tile([M, 128], F32, tag="ps")
                        first = True
                        for q in range(NQ):
                            for j in range(B):
                                ch = wl * B + j
                                if attn:
                                    lhsT = Us[q][:, ch, :]
                                else:
                                    lhsT = lands[q][:, ch, c0:c1]
                                nc.tensor.matmul(
                                    ps[:, :],
                                    lhsT,
                                    Ss[q][:, ch, :],
                                    start=first,
                                    stop=(q == NQ - 1 and j == B - 1),
                                )
                                first = False
                        nc.vector.tensor_copy(
                            out=stage[:, wl * 128:(wl + 1) * 128], in_=ps[:, :])
                    nc.sync.dma_start(
                        out=outs[t][:, sg * SG * 128:(sg + 1) * SG * 128],
                        in_=stage[:, :])

    nc.finalize()
    return nc


def kernel(**inputs):
    global _PROGRAM
    inp = {k: np.asarray(v) for k, v in inputs.items()}

    s_feat = inp["s_feat"].astype(np.float32)
    o_feat = inp["o_feat"].astype(np.float32)
    Wss_w, Wss_b = inp["Wss_w"].astype(np.float32), inp["Wss_b"].astype(np.float32)
    Wos_w, Wos_b = inp["Wos_w"].astype(np.float32), inp["Wos_b"].astype(np.float32)
    Ws_w, Ws_b = inp["Ws_w"].astype(np.float32), inp["Ws_b"].astype(np.float32)
    attn_w, attn_b = inp["attn_w"].astype(np.float32), inp["attn_b"].astype(np.float32)
    Win_w, Win_b = inp["Win_w"].astype(np.float32), inp["Win_b"].astype(np.float32)
    Wself_w, Wself_b = inp["Wself_w"].astype(np.float32), inp["Wself_b"].astype(np.float32)
    Wout_w, Wout_b = inp["Wout_w"].astype(np.float32), inp["Wout_b"].astype(np.float32)
    Wo_w, Wo_b = inp["Wo_w"].astype(np.float32), inp["Wo_b"].astype(np.float32)

    aw1 = attn_w[:D, 0]
    aw2 = attn_w[D:, 0]

    m_ss = s_feat @ Wss_w[:D]
    qm_ss = m_ss @ aw1
    m_os = o_feat @ Wos_w[:D]
    qm_os = m_os @ aw1
    t_in = o_feat @ Win_w + Win_b
    t_out = o_feat @ Wout_w + Wout_b

    t_s = np.zeros((N_S, 128), np.float16)
    t_s[:, 0:64] = m_ss
    t_s[:, 64] = qm_ss
    t_o = np.zeros((N_O, 128), np.float16)
    t_o[:, 0:64] = m_os
    t_o[:, 64] = qm_os
    t_x = np.zeros((N_O, 128), np.float16)
    t_x[:, 0:64] = t_in
    t_x[:, 64:128] = t_out

    h_s = s_feat @ Ws_w + Ws_b
    a2 = h_s @ aw2

    W2ss = Wss_w[D:]
    W2os = Wos_w[D:]
    ef_ss = inp["efeat_ss"].astype(np.float32)
    ef_os = inp["efeat_os"].astype(np.float32)
    c_ss_edge = ef_ss @ (W2ss @ aw1) + (Wss_b @ aw1 + attn_b[0]) + a2[inp["ss_dst"]]
    c_os_edge = ef_os @ (W2os @ aw1) + (Wos_b @ aw1 + attn_b[0]) + a2[inp["os_dst"]]

    edge_cfg = {
        "ss": (inp["ss_src"], inp["ss_dst"], c_ss_edge, ef_ss, 10, True),
        "os": (inp["os_src"], inp["os_dst"], c_os_edge, ef_os, 2, True),
        "fw": (inp["fwd_src"], inp["fwd_dst"], None, None, 0, False),
        "bw": (inp["bwd_src"], inp["bwd_dst"], None, None, 0, False),
    }

    in_maps = [dict() for _ in range(NC)]
    iota = np.tile(np.arange(128, dtype=np.float16)[None, None, :], (128, 1, 1))
    for c in range(NC):
        in_maps[c]["t_s"] = t_s
        in_maps[c]["t_o"] = t_o
        in_maps[c]["t_x"] = t_x
        in_maps[c]["iota"] = iota

    overflow = {}
    for t, (src, dst, c_e, ef, ext, attn) in edge_cfg.items():
        src = np.asarray(src).astype(np.int64)
        dst = np.asarray(dst).astype(np.int64)
        core = dst // SHARD
        ldst = dst - core * SHARD
        w = ldst // WIN
        drel = (ldst - w * WIN).astype(np.float16)
        q = src // QD
        lsrc = (src - q * QD).astype(np.int16)
        sg = w // SG
        wl = w - sg * SG
        gid = ((core * NSG + sg) * NQ + q) * SG + wl
        NG = NC * NSG * NQ * SG
        order = np.argsort(gid, kind="stable")
        cnt = np.bincount(gid, minlength=NG)
        starts = np.zeros(NG + 1, np.int64)
        np.cumsum(cnt, out=starts[1:])
        rank = np.empty(E, np.int64)
        rank[order] = np.arange(E) - starts[gid[order]]
        ok = rank < SLOTS_WQ
        if not ok.all():
            overflow[t] = np.where(~ok)[0]
        tok = wl * SLOTS_WQ + rank

        idx_a = np.zeros((NC, NSG, NQ, TOK), np.int16)
        dr_a = np.full((NC, NSG, NQ, TOK), -1.0, np.float16)
        idx_a[core[ok], sg[ok], q[ok], tok[ok]] = lsrc[ok]
        dr_a[core[ok], sg[ok], q[ok], tok[ok]] = drel[ok]
        idx_w = idx_a.reshape(NC, NSG, NQ, TOK // 16, 16).transpose(0, 1, 2, 4, 3)
        idx_w = np.broadcast_to(idx_w[:, :, :, None, :, :],
                                (NC, NSG, NQ, 8, 16, TOK // 16))
        idx_w = np.ascontiguousarray(idx_w).reshape(NC, NSG, NQ, 128, TOK // 16)
        dr_w = dr_a.reshape(NC, NSG, NQ, CHUNKS_SG, 128).transpose(0, 1, 2, 4, 3)
        for c in range(NC):
            in_maps[c][f"idx_{t}"] = np.ascontiguousarray(idx_w[c].transpose(1, 0, 2, 3))
            in_maps[c][f"dr_{t}"] = np.ascontiguousarray(
                dr_w[c].transpose(1, 0, 2, 3))[:, :, :, :, None]
        if attn:
            cc_a = np.zeros((NC, NSG, NQ, TOK), np.float32)
            cc_a[core[ok], sg[ok], q[ok], tok[ok]] = c_e[ok]
            cc_w = cc_a.reshape(NC, NSG, NQ, CHUNKS_SG, 128).transpose(0, 1, 2, 4, 3)
            ef_a = np.zeros((NC, NSG, NQ, TOK, ext), np.float16)
            ef_a[core[ok], sg[ok], q[ok], tok[ok]] = ef[ok]
            ef_w = ef_a.reshape(NC, NSG, NQ, CHUNKS_SG, 128, ext).transpose(0, 1, 2, 4, 3, 5)
            for c in range(NC):
                in_maps[c][f"c_{t}"] = np.ascontiguousarray(
                    cc_w[c].transpose(1, 0, 2, 3))[:, :, :, :, None]
                in_maps[c][f"ef_{t}"] = np.ascontiguousarray(
                    ef_w[c].transpose(1, 0, 2, 3, 4))

    if _PROGRAM is None:
        _PROGRAM = _build_program()
    import time as _time
    _t0 = _time.time()
    res = run_bass_kernel_spmd(_PROGRAM, in_maps, list(range(NC)))
    global LAST_DEVICE_WALL_NS
    LAST_DEVICE_WALL_NS = (_time.time() - _t0) * 1e9

    A_ss = np.concatenate([res.results[c]["A_ss"][:, :SHARD] for c in range(NC)], axis=1)
    A_os = np.concatenate([res.results[c]["A_os"][:, :SHARD] for c in range(NC)], axis=1)
    A_fw = np.concatenate([res.results[c]["A_fw"][:, :SHARD] for c in range(NC)], axis=1)
    A_bw = np.concatenate([res.results[c]["A_bw"][:, :SHARD] for c in range(NC)], axis=1)

    def corr_attn(t, A, src, dst, c_e, ef, tbl):
        idxs = overflow.get(t)
        if idxs is None:
            return
        s = np.asarray(src)[idxs].astype(np.int64)
        d = np.asarray(dst)[idxs].astype(np.int64)
        m = tbl[s, 0:64].astype(np.float32)
        qm = tbl[s, 64].astype(np.float32)
        sc = qm + c_e[idxs]
        sc = np.maximum(sc, 0.01 * sc)
        nom = np.exp(sc)
        np.add.at(A.T, d, np.concatenate(
            [m * nom[:, None], ef[idxs] * nom[:, None], nom[:, None]], axis=1))

    def corr_plain(t, A, src, dst, tbl, cols):
        idxs = overflow.get(t)
        if idxs is None:
            return
        s = np.asarray(src)[idxs].astype(np.int64)
        d = np.asarray(dst)[idxs].astype(np.int64)
        np.add.at(A.T, d, tbl[s, cols[0]:cols[1]].astype(np.float32))

    corr_attn("ss", A_ss, inp["ss_src"], inp["ss_dst"], c_ss_edge, ef_ss, t_s)
    corr_attn("os", A_os, inp["os_src"], inp["os_dst"], c_os_edge, ef_os, t_o)
    corr_plain("fw", A_fw, inp["fwd_src"], inp["fwd_dst"], t_x, (0, 64))
    corr_plain("bw", A_bw, inp["bwd_src"], inp["bwd_dst"], t_x, (64, 128))

    def z_part(A, W2, bvec):
        den = A[-1]
        numT = A[0:64] + W2.T @ A[64:-1] + np.outer(bvec, den)
        safe = np.where(den == 0, 1.0, den)
        return np.where(den[None, :] > 0, numT / safe[None, :], 0.0)

    zT = z_part(A_ss, W2ss, Wss_b) + z_part(A_os, W2os, Wos_b)
    z = np.ascontiguousarray(zT.T, dtype=np.float32)

    h_self = o_feat @ Wself_w + Wself_b
    x = (np.maximum(A_fw.T, 0) @ Wo_w[0:64]
         + np.maximum(h_self, 0) @ Wo_w[64:128]
         + np.maximum(A_bw.T, 0) @ Wo_w[128:192]
         + Wo_b).astype(np.float32)

    return (z, x)



# revision 2
# speedup vs baseline: 1.7180x; 1.7180x over previous
"""Trainium2 Bass kernel for nn_AttnConvLayer (GNN message passing), v2.

Wire-optimized vs v1: inputs ~97MB total (vs ~740MB), outputs 25.6MB
(vs 108MB), everything else moved on-device:
  - s_feat/o_feat shipped fp16, feature-major, sharded 1/8 per core;
    AllGather on device; the three 256B-row gather tables (t_s =
    [m_ss|qm_ss], t_o = [m_os|qm_os], t_x = [t_in|t_out]) are built on
    device with PE matmuls + PE transposes.
  - Tables padded per-core-segment to 12544 rows so quadrant-local
    gather indices fit int16 (4 quadrants x 25088 rows).
  - Edge shards: dst-sharded per core, 128-node dst windows, budget
    512 slots per (window, quadrant) -> overflow probability ~1e-17.
  - idx shipped compact [16, .] and replicated to 128 partitions on
    device; dr/c/ef fp16.
  - Full finalize on device: per-dst softmax normalize + W2 fold for z,
    relu/Wo combine + h_self for x; output [2, 12544, 64] fp16/core.
"""

import sys
sys.path.insert(0, '/opt/trn_rl_repo')
import numpy as np

N_S = 100000
N_O = 100000
D = 64
NC = 8
SHARD = N_S // NC          # 12500
WIN = 128
NWIN = 98                  # ceil(12500/128)
NODES = NWIN * WIN         # 12544 padded nodes per core
SG = 7                     # windows per supergroup
NSG = NWIN // SG           # 14
HALF = NWIN // 2           # 49 windows per finalize half
B = 4                      # chunks per (window, quadrant)
SLOTS_W = B * WIN          # 512
CHUNKS_SG = SG * B         # 28
TOK = SG * SLOTS_W         # 3584 tokens per (sg, q) gather
NQ = 4
QROWS = 2 * NODES          # 25088 table rows per quadrant
TROWS = NC * NODES         # 100352

# name, ext, attn, table idx, col0, M
TYPES = [
    ("ss", 10, True, 0, 0, 75),
    ("os", 2, True, 1, 0, 67),
    ("fw", 0, False, 2, 0, 64),
    ("bw", 0, False, 2, 64, 64),
]

_PROGRAM = None
LAST_DEVICE_WALL_NS = None


# ---------------------------------------------------------------- host pack

def _pack(inp):
    f16 = np.float16
    s_feat = inp["s_feat"].astype(np.float32)
    o_feat = inp["o_feat"].astype(np.float32)
    Wss_w, Wss_b = inp["Wss_w"].astype(np.float32), inp["Wss_b"].astype(np.float32)
    Wos_w, Wos_b = inp["Wos_w"].astype(np.float32), inp["Wos_b"].astype(np.float32)
    Ws_w, Ws_b = inp["Ws_w"].astype(np.float32), inp["Ws_b"].astype(np.float32)
    attn_w, attn_b = inp["attn_w"].astype(np.float32), inp["attn_b"].astype(np.float32)
    Win_w, Win_b = inp["Win_w"].astype(np.float32), inp["Win_b"].astype(np.float32)
    Wself_w, Wself_b = inp["Wself_w"].astype(np.float32), inp["Wself_b"].astype(np.float32)
    Wout_w, Wout_b = inp["Wout_w"].astype(np.float32), inp["Wout_b"].astype(np.float32)
    Wo_w, Wo_b = inp["Wo_w"].astype(np.float32), inp["Wo_b"].astype(np.float32)

    aw1 = attn_w[:D, 0]
    aw2 = attn_w[D:, 0]
    W2ss = Wss_w[D:]     # [10, 64]
    W2os = Wos_w[D:]     # [2, 64]

    # ---- feature shards, feature-major fp16, padded to NODES cols ----
    sT = np.ascontiguousarray(s_feat.T.astype(f16))    # [64, N_S]
    oT = np.ascontiguousarray(o_feat.T.astype(f16))
    feat = np.zeros((NC, 2, D, NODES), f16)
    feat[:, 0, :, :SHARD] = sT.reshape(D, NC, SHARD).transpose(1, 0, 2)
    feat[:, 1, :, :SHARD] = oT.reshape(D, NC, SHARD).transpose(1, 0, 2)

    # ---- small weights ----
    wc = np.zeros((D, 3, 128), f16)
    wc[:, 0, 0:64] = Wss_w[:D]
    wc[:, 0, 64] = Wss_w[:D] @ aw1
    wc[:, 1, 0:64] = Wos_w[:D]
    wc[:, 1, 64] = Wos_w[:D] @ aw1
    wc[:, 2, 0:64] = Win_w
    wc[:, 2, 64:128] = Wout_w
    biasx = np.concatenate([Win_b, Wout_b]).astype(np.float32)[:, None]  # [128,1]

    w2a_ss = np.zeros((75, 64), f16)
    w2a_ss[64] = Wss_b
    w2a_ss[65:75] = W2ss
    w2a_os = np.zeros((67, 64), f16)
    w2a_os[64] = Wos_b
    w2a_os[65:67] = W2os

    wfin = np.zeros((D, 4, 64), f16)
    wfin[:, 0, :] = Wo_w[0:64]      # h_in path
    wfin[:, 1, :] = Wo_w[64:128]    # h_self path
    wfin[:, 2, :] = Wo_w[128:192]   # h_out path
    wfin[:, 3, :] = Wself_w
    bfin = np.zeros((D, 2, 1), np.float32)
    bfin[:, 0, 0] = Wo_b
    bfin[:, 1, 0] = Wself_b

    iden = np.eye(128, dtype=f16)
    iota = np.tile(np.arange(128, dtype=f16)[None, None, :], (128, 1, 1))

    # ---- attention edge constants (host) ----
    h_s = s_feat @ Ws_w + Ws_b
    a2 = h_s @ aw2
    ef_ss = inp["efeat_ss"].astype(np.float32)
    ef_os = inp["efeat_os"].astype(np.float32)
    c_ss = ef_ss @ (W2ss @ aw1) + (Wss_b @ aw1 + attn_b[0]) \
        + a2[np.asarray(inp["ss_dst"]).astype(np.int64)]
    c_os = ef_os @ (W2os @ aw1) + (Wos_b @ aw1 + attn_b[0]) \
        + a2[np.asarray(inp["os_dst"]).astype(np.int64)]

    edge_cfg = {
        "ss": (inp["ss_src"], inp["ss_dst"], c_ss, ef_ss, 10),
        "os": (inp["os_src"], inp["os_dst"], c_os, ef_os, 2),
        "fw": (inp["fwd_src"], inp["fwd_dst"], None, None, 0),
        "bw": (inp["bwd_src"], inp["bwd_dst"], None, None, 0),
    }

    in_maps = [dict() for _ in range(NC)]
    for c in range(NC):
        in_maps[c]["feat"] = feat[c]
        in_maps[c]["wc"] = wc
        in_maps[c]["biasx"] = biasx
        in_maps[c]["w2a_ss"] = w2a_ss
        in_maps[c]["w2a_os"] = w2a_os
        in_maps[c]["wfin"] = wfin
        in_maps[c]["bfin"] = bfin
        in_maps[c]["iden"] = iden
        in_maps[c]["iota"] = iota

    for t, (src, dst, c_e, ef, ext) in edge_cfg.items():
        src = np.asarray(src).astype(np.int64)
        dst = np.asarray(dst).astype(np.int64)
        E = src.shape[0]
        core = dst // SHARD
        ldst = dst - core * SHARD
        w = ldst // WIN
        drel = (ldst - w * WIN).astype(f16)
        r = (src // SHARD) * NODES + (src - (src // SHARD) * SHARD)
        q = r // QROWS
        lsrc = (r - q * QROWS).astype(np.int16)
        sg = w // SG
        wl = w - sg * SG

        gid = ((core * NWIN + w) * NQ + q)
        NG = NC * NWIN * NQ
        order = np.argsort(gid, kind="stable")
        cnt = np.bincount(gid, minlength=NG)
        starts = np.zeros(NG + 1, np.int64)
        np.cumsum(cnt, out=starts[1:])
        rank = np.empty(E, np.int64)
        rank[order] = np.arange(E) - starts[gid[order]]
        if not (rank < SLOTS_W).all():
            # ~1e-17 probability; drop excess edges rather than crash
            keep = rank < SLOTS_W
            src, dst, core, sg, wl, q, lsrc, drel, rank = (
                a[keep] for a in (src, dst, core, sg, wl, q, lsrc, drel, rank))
            if c_e is not None:
                c_e, ef = c_e[keep], ef[keep]
        tok = wl * SLOTS_W + rank

        idx_a = np.zeros((NC, NSG, NQ, TOK), np.int16)
        dr_a = np.full((NC, NSG, NQ, TOK), -1.0, f16)
        idx_a[core, sg, q, tok] = lsrc
        dr_a[core, sg, q, tok] = drel
        # device layouts
        idx_w = np.ascontiguousarray(
            idx_a.reshape(NC, NSG, NQ, TOK // 16, 16).transpose(0, 1, 4, 2, 3))
        # [NC, NSG, 16, NQ, 224]
        dr_w = np.ascontiguousarray(
            dr_a.reshape(NC, NSG, NQ, CHUNKS_SG, 128).transpose(0, 1, 4, 2, 3))
        # [NC, NSG, 128, NQ, 28]
        for c in range(NC):
            in_maps[c][f"idx_{t}"] = idx_w[c]
            in_maps[c][f"dr_{t}"] = dr_w[c]
        if c_e is not None:
            cc_a = np.zeros((NC, NSG, NQ, TOK), f16)
            cc_a[core, sg, q, tok] = c_e.astype(f16)
            cc_w = np.ascontiguousarray(
                cc_a.reshape(NC, NSG, NQ, CHUNKS_SG, 128).transpose(0, 1, 4, 2, 3))
            ef_a = np.zeros((NC, NSG, NQ, TOK, ext), f16)
            ef_a[core, sg, q, tok] = ef.astype(f16)
            ef_w = np.ascontiguousarray(
                ef_a.reshape(NC, NSG, NQ, CHUNKS_SG, 128, ext)
                .transpose(0, 1, 4, 2, 3, 5))
            for c in range(NC):
                in_maps[c][f"c_{t}"] = cc_w[c]
                in_maps[c][f"ef_{t}"] = ef_w[c]
    return in_maps


# ---------------------------------------------------------------- bass build

def _build_program():
    from concourse import bass, bacc, mybir
    import concourse.tile as tile

    F16 = mybir.dt.float16
    F32 = mybir.dt.float32
    I16 = mybir.dt.int16
    AF = mybir.ActivationFunctionType
    OP = mybir.AluOpType

    nc = bacc.Bacc(None, target_bir_lowering=False, num_devices=NC,
                   dynamic_dma_scratch_size=2 ** 15)

    inp = {}
    inp["feat"] = nc.declare_dram_parameter("feat", [2, D, NODES], F16, isOutput=False)
    inp["wc"] = nc.declare_dram_parameter("wc", [D, 3, 128], F16, isOutput=False)
    inp["biasx"] = nc.declare_dram_parameter("biasx", [128, 1], F32, isOutput=False)
    inp["w2a_ss"] = nc.declare_dram_parameter("w2a_ss", [75, 64], F16, isOutput=False)
    inp["w2a_os"] = nc.declare_dram_parameter("w2a_os", [67, 64], F16, isOutput=False)
    inp["wfin"] = nc.declare_dram_parameter("wfin", [D, 4, 64], F16, isOutput=False)
    inp["bfin"] = nc.declare_dram_parameter("bfin", [D, 2, 1], F32, isOutput=False)
    inp["iden"] = nc.declare_dram_parameter("iden", [128, 128], F16, isOutput=False)
    inp["iota"] = nc.declare_dram_parameter("iota", [128, 1, 128], F16, isOutput=False)
    for t, ext, attn, _, _, _ in TYPES:
        inp[f"idx_{t}"] = nc.declare_dram_parameter(
            f"idx_{t}", [NSG, 16, NQ, TOK // 16], I16, isOutput=False)
        inp[f"dr_{t}"] = nc.declare_dram_parameter(
            f"dr_{t}", [NSG, 128, NQ, CHUNKS_SG], F16, isOutput=False)
        if attn:
            inp[f"c_{t}"] = nc.declare_dram_parameter(
                f"c_{t}", [NSG, 128, NQ, CHUNKS_SG], F16, isOutput=False)
            inp[f"ef_{t}"] = nc.declare_dram_parameter(
                f"ef_{t}", [NSG, 128, NQ, CHUNKS_SG, ext], F16, isOutput=False)
    out = nc.declare_dram_parameter("out", [2, NODES, D], F16, isOutput=True)

    with tile.TileContext(nc) as tc:
        with tc.tile_pool(name="dram", bufs=1, space="DRAM") as dram:
            bounce = dram.tile([2, D, NODES], F16)
            featg = dram.tile([NC, 2, D, NODES], F16, addr_space="Shared")
            tbl = dram.tile([3, TROWS, 128], F16)

            nc.sync.dma_start(out=bounce[:, :, :], in_=inp["feat"][:, :, :])
            nc.gpsimd.collective_compute(
                "AllGather", OP.bypass,
                replica_groups=[list(range(NC))],
                ins=[bounce[:, :, :].opt()],
                outs=[featg[:, :, :, :].opt()],
            )

            # ---------------- phase A: build tables ----------------
            with (
                tc.tile_pool(name="tconst", bufs=1) as tcp,
                tc.tile_pool(name="tbuild", bufs=3) as tp,
                tc.tile_pool(name="tpsum", bufs=2, space="PSUM") as tpp,
            ):
                wc_sb = tcp.tile([D, 3, 128], F16)
                nc.sync.dma_start(out=wc_sb[:, :, :], in_=inp["wc"][:, :, :])
                bx_sb = tcp.tile([128, 1], F32)
                nc.sync.dma_start(out=bx_sb[:, :], in_=inp["biasx"][:, :])
                idA_sb = tcp.tile([128, 128], F16)
                nc.sync.dma_start(out=idA_sb[:, :], in_=inp["iden"][:, :])

                for c8 in range(NC):
                    for t in range(3):
                        srcf = 0 if t == 0 else 1
                        for j0 in range(0, NODES, 512):
                            W = min(512, NODES - j0)
                            KT = W // 128
                            rsb = tp.tile([D, 512], F16, tag="rsb")
                            nc.sync.dma_start(
                                out=rsb[:, :W], in_=featg[c8, srcf, :, j0:j0 + W])
                            ps = tpp.tile([128, 512], F32, tag="psA")
                            nc.tensor.matmul(ps[:, :W], wc_sb[:, t, :], rsb[:, :W],
                                             start=True, stop=True)
                            csb = tp.tile([128, 512], F16, tag="csb")
                            if t == 2:
                                nc.vector.tensor_tensor(
                                    out=csb[:, :W], in0=ps[:, :W],
                                    in1=bx_sb[:, :].to_broadcast([128, W]),
                                    op=OP.add)
                            else:
                                nc.scalar.activation(csb[:, :W], ps[:, :W], AF.Copy)
                            ps2 = tpp.tile([128, 4, 128], F32, tag="psA2")
                            for k in range(KT):
                                nc.tensor.matmul(
                                    ps2[:, k, :], csb[:, k * 128:(k + 1) * 128],
                                    idA_sb[:, :], start=True, stop=True)
                            osb = tp.tile([128, 4, 128], F16, tag="osb")
                            if (j0 // 512) % 2 == 0:
                                nc.vector.tensor_copy(out=osb[:, :KT, :],
                                                      in_=ps2[:, :KT, :])
                            else:
                                nc.scalar.activation(osb[:, :KT, :], ps2[:, :KT, :],
                                                     AF.Copy)
                            base = c8 * NODES + j0
                            nc.sync.dma_start(
                                out=tbl[t, base:base + W, :].rearrange(
                                    "(k p) f -> p k f", p=128),
                                in_=osb[:, :KT, :])

            # ---------------- phase B: edges + finalize ----------------
            with (
                tc.tile_pool(name="const", bufs=1) as cp,
                tc.tile_pool(name="acc", bufs=1) as ap_,
                tc.tile_pool(name="work", bufs=2) as wp,
                tc.tile_pool(name="small", bufs=3) as sp,
                tc.tile_pool(name="eps", bufs=3, space="PSUM") as epp,
                tc.tile_pool(name="fin", bufs=1, space="PSUM") as fpp,
            ):
                iden_sb = cp.tile([128, 128], F16)
                nc.sync.dma_start(out=iden_sb[:, :], in_=inp["iden"][:, :])
                iota_sb = cp.tile([128, 1, 128], F16)
                nc.sync.dma_start(out=iota_sb[:, :, :], in_=inp["iota"][:, :, :])
                w2ss_sb = cp.tile([75, 64], F16)
                nc.sync.dma_start(out=w2ss_sb[:, :], in_=inp["w2a_ss"][:, :])
                w2os_sb = cp.tile([67, 64], F16)
                nc.sync.dma_start(out=w2os_sb[:, :], in_=inp["w2a_os"][:, :])
                wfin_sb = cp.tile([D, 4, 64], F16)
                nc.sync.dma_start(out=wfin_sb[:, :, :], in_=inp["wfin"][:, :, :])
                bfin_sb = cp.tile([D, 2, 1], F32)
                nc.sync.dma_start(out=bfin_sb[:, :, :], in_=inp["bfin"][:, :, :])

                for half in range(2):
                    accs = {}
                    for (tname, ext, attn, tq, col0, M) in TYPES:
                        acc = ap_.tile([75, HALF, 128], F16, tag=f"acc_{tname}")
                        accs[tname] = acc
                        for sgl in range(NSG // 2):
                            sg = half * (NSG // 2) + sgl
                            idx_sb = wp.tile([128, NQ, TOK // 16], I16, tag="idx")
                            for k in range(8):
                                nc.sync.dma_start(
                                    out=idx_sb[16 * k:16 * (k + 1), :, :],
                                    in_=inp[f"idx_{tname}"][sg, :, :, :])
                            dr_sb = wp.tile([128, NQ, CHUNKS_SG], F16, tag="dr")
                            nc.sync.dma_start(out=dr_sb[:, :, :],
                                              in_=inp[f"dr_{tname}"][sg, :, :, :])
                            land = wp.tile([128, NQ, CHUNKS_SG, 128], F16, tag="land")
                            for q in range(NQ):
                                nc.gpsimd.dma_gather(
                                    out_ap=land[:, q, :, :],
                                    in_ap=tbl[tq, q * QROWS:(q + 1) * QROWS, :],
                                    idxs_ap=idx_sb[:, q, :],
                                    num_idxs=TOK,
                                    num_idxs_reg=TOK,
                                    elem_size=128,
                                    single_packet=False,
                                )
                            if attn:
                                c_sb = wp.tile([128, NQ, CHUNKS_SG], F16, tag="cc")
                                nc.sync.dma_start(out=c_sb[:, :, :],
                                                  in_=inp[f"c_{tname}"][sg, :, :, :])
                                ef_sb = wp.tile([128, NQ, CHUNKS_SG, 10], F16,
                                                tag="ef")
                                nc.sync.dma_start(
                                    out=ef_sb[:, :, :, :ext],
                                    in_=inp[f"ef_{tname}"][sg, :, :, :, :])
                                sv = wp.tile([128, NQ, CHUNKS_SG, 1], F32, tag="sv")
                                nc.vector.tensor_tensor(
                                    out=sv[:, :, :, :], in0=land[:, :, :, 64:65],
                                    in1=c_sb[:, :, :].unsqueeze(3), op=OP.add)
                                nc.scalar.activation(sv[:, :, :, :], sv[:, :, :, :],
                                                     AF.Lrelu, alpha=0.01)
                                nom = wp.tile([128, NQ, CHUNKS_SG, 1], F16, tag="nom")
                                nc.scalar.activation(nom[:, :, :, :], sv[:, :, :, :],
                                                     AF.Exp)
                                U = wp.tile([128, NQ, CHUNKS_SG, 75], F16, tag="U")
                                nc.vector.tensor_tensor(
                                    out=U[:, :, :, 0:64], in0=land[:, :, :, 0:64],
                                    in1=nom[:, :, :, :].to_broadcast(
                                        [128, NQ, CHUNKS_SG, 64]),
                                    op=OP.mult)
                                nc.vector.tensor_tensor(
                                    out=U[:, :, :, 65:65 + ext],
                                    in0=ef_sb[:, :, :, :ext],
                                    in1=nom[:, :, :, :].to_broadcast(
                                        [128, NQ, CHUNKS_SG, ext]),
                                    op=OP.mult)
                                nc.scalar.activation(
                                    U[:, :, :, 64:65], nom[:, :, :, :], AF.Copy)
                            for wl in range(SG):
                                S = sp.tile([128, NQ, B, 128], F16, tag="S")
                                nc.vector.tensor_tensor(
                                    out=S[:, :, :, :],
                                    in0=dr_sb[:, :, wl * B:(wl + 1) * B]
                                    .unsqueeze(3).to_broadcast([128, NQ, B, 128]),
                                    in1=iota_sb[:, 0:1, :].unsqueeze(1)
                                    .to_broadcast([128, NQ, B, 128]),
                                    op=OP.is_equal)
                                ps = epp.tile([75, 128], F32, tag="eps")
                                for q in range(NQ):
                                    for j in range(B):
                                        ch = wl * B + j
                                        if attn:
                                            lhsT = U[:, q, ch, 0:M]
                                        else:
                                            lhsT = land[:, q, ch, col0:col0 + 64]
                                        nc.tensor.matmul(
                                            ps[0:M, :], lhsT, S[:, q, j, :],
                                            start=(q == 0 and j == 0),
                                            stop=(q == NQ - 1 and j == B - 1))
                                wloc = sgl * SG + wl
                                nc.vector.tensor_copy(out=acc[0:M, wloc, :],
                                                      in_=ps[0:M, :])
                    # ---- finalize this half ----
                    for wloc in range(HALF):
                        n0 = (half * HALF + wloc) * 128
                        a_ss, a_os = accs["ss"], accs["os"]
                        a_fw, a_bw = accs["fw"], accs["bw"]
                        nm = fpp.tile([64, 2, 128], F32, tag="nm")
                        nc.tensor.matmul(nm[:, 0, :], iden_sb[0:64, 0:64],
                                         a_ss[0:64, wloc, :], start=True, stop=False)
                        nc.tensor.matmul(nm[:, 0, :], w2ss_sb[64:75, :],
                                         a_ss[64:75, wloc, :], start=False, stop=True)
                        nc.tensor.matmul(nm[:, 1, :], iden_sb[0:64, 0:64],
                                         a_os[0:64, wloc, :], start=True, stop=False)
                        nc.tensor.matmul(nm[:, 1, :], w2os_sb[64:67, :],
                                         a_os[64:67, wloc, :], start=False, stop=True)
                        dT = fpp.tile([128, 2, 1], F32, tag="dT")
                        nc.tensor.matmul(dT[:, 0, :], a_ss[64:65, wloc, :],
                                         iden_sb[64:65, 64:65], start=True, stop=True)
                        nc.tensor.matmul(dT[:, 1, :], a_os[64:65, wloc, :],
                                         iden_sb[64:65, 64:65], start=True, stop=True)
                        dmx = sp.tile([128, 2, 1], F32, tag="dmx")
                        nc.vector.tensor_scalar_max(dmx[:, :, :], dT[:, :, :], 1e-20)
                        rec = sp.tile([128, 2, 1], F32, tag="rec")
                        nc.vector.reciprocal(rec[:, :, :], dmx[:, :, :])
                        nsb = sp.tile([64, 2, 128], F16, tag="nsb")
                        nc.scalar.activation(nsb[:, :, :], nm[:, :, :], AF.Copy)
                        zT = fpp.tile([128, 2, 64], F32, tag="zT")
                        nc.tensor.matmul(zT[:, 0, :], nsb[:, 0, :],
                                         iden_sb[0:64, 0:64], start=True, stop=True)
                        nc.tensor.matmul(zT[:, 1, :], nsb[:, 1, :],
                                         iden_sb[0:64, 0:64], start=True, stop=True)
                        zp = sp.tile([128, 2, 64], F32, tag="zp")
                        nc.vector.tensor_scalar(
                            out=zp[:, 0, :], in0=zT[:, 0, :],
                            scalar1=rec[:, 0, :], scalar2=None, op0=OP.mult)
                        nc.vector.tensor_scalar(
                            out=zp[:, 1, :], in0=zT[:, 1, :],
                            scalar1=rec[:, 1, :], scalar2=None, op0=OP.mult)
                        zo = sp.tile([128, 64], F16, tag="zo")
                        nc.vector.tensor_tensor(out=zo[:, :], in0=zp[:, 0, :],
                                                in1=zp[:, 1, :], op=OP.add)
                        nc.sync.dma_start(out=out[0, n0:n0 + 128, :], in_=zo[:, :])
                        # x path
                        fsb = sp.tile([64, 128], F16, tag="fsb")
                        nc.sync.dma_start(out=fsb[:, :],
                                          in_=inp["feat"][1, :, n0:n0 + 128])
                        hx = fpp.tile([64, 2, 128], F32, tag="hx")
                        nc.tensor.matmul(hx[:, 0, :], wfin_sb[:, 3, :], fsb[:, :],
                                         start=True, stop=True)
                        rl = sp.tile([64, 3, 128], F16, tag="rl")
                        nc.scalar.activation(rl[:, 0, :], a_fw[0:64, wloc, :],
                                             AF.Relu)
                        nc.scalar.activation(rl[:, 1, :], hx[:, 0, :], AF.Relu,
                                             bias=bfin_sb[:, 1, :])
                        nc.scalar.activation(rl[:, 2, :], a_bw[0:64, wloc, :],
                                             AF.Relu)
                        nc.tensor.matmul(hx[:, 1, :], wfin_sb[:, 0, :], rl[:, 0, :],
                                         start=True, stop=False)
                        nc.tensor.matmul(hx[:, 1, :], wfin_sb[:, 1, :], rl[:, 1, :],
                                         start=False, stop=False)
                        nc.tensor.matmul(hx[:, 1, :], wfin_sb[:, 2, :], rl[:, 2, :],
                                         start=False, stop=True)
                        xsb = sp.tile([64, 128], F16, tag="xsb")
                        nc.vector.tensor_tensor(
                            out=xsb[:, :], in0=hx[:, 1, :],
                            in1=bfin_sb[:, 0, :].to_broadcast([64, 128]), op=OP.add)
                        xT = fpp.tile([128, 64], F32, tag="xT")
                        nc.tensor.matmul(xT[:, :], xsb[:, :], iden_sb[0:64, 0:64],
                                         start=True, stop=True)
                        xo = sp.tile([128, 64], F16, tag="xo")
                        nc.scalar.activation(xo[:, :], xT[:, :], AF.Copy)
                        nc.sync.dma_start(out=out[1, n0:n0 + 128, :], in_=xo[:, :])

    nc.finalize()
    return nc


# ---------------------------------------------------------------- numpy sim

def _simulate(in_maps):
    """Numpy emulation of the device program (fp16 rounding where it
    matters) — validates packing + math without compiling."""
    f16 = np.float16
    results = []
    for c in range(NC):
        results.append({})
    # AllGather
    featg = np.stack([in_maps[c]["feat"] for c in range(NC)])  # [NC,2,64,NODES]
    wc = in_maps[0]["wc"].astype(np.float32)
    biasx = in_maps[0]["biasx"].astype(np.float32)
    # tables (same on all cores)
    tbl = np.zeros((3, TROWS, 128), f16)
    for t in range(3):
        srcf = 0 if t == 0 else 1
        ft = featg[:, srcf].transpose(0, 2, 1).reshape(TROWS, D)  # [TROWS, 64]
        m = ft.astype(np.float32) @ wc[:, t, :]
        if t == 2:
            m = m + biasx[:, 0][None, :]
        tbl[t] = m.astype(f16)

    for c in range(NC):
        im = in_maps[c]
        out = np.zeros((2, NODES, D), f16)
        acc_all = {}
        for (tname, ext, attn, tq, col0, M) in TYPES:
            acc = np.zeros((M, NWIN, 128), f16)
            for sg in range(NSG):
                idx = im[f"idx_{tname}"][sg]      # [16, NQ, 224]
                dr = im[f"dr_{tname}"][sg]        # [128, NQ, 28]
                # reconstruct tokens: token t at [t%16, q, t//16]
                toks = idx.transpose(1, 2, 0).reshape(NQ, TOK)  # [NQ, TOK]
                land = np.zeros((128, NQ, CHUNKS_SG, 128), f16)
                for q in range(NQ):
                    g = tbl[tq, q * QROWS + toks[q].astype(np.int64), :]
                    land[:, q, :, :] = g.reshape(CHUNKS_SG, 128, 128).transpose(1, 0, 2)
                if attn:
                    cc = im[f"c_{tname}"][sg]     # [128, NQ, 28]
                    ef = im[f"ef_{tname}"][sg]    # [128, NQ, 28, ext]
                    sv = land[:, :, :, 64].astype(np.float32) + cc.astype(np.float32)
                    sv = np.where(sv > 0, sv, 0.01 * sv)
                    nom = np.exp(sv).astype(f16)
                    U = np.zeros((128, NQ, CHUNKS_SG, M), f16)
                    U[..., 0:64] = (land[..., 0:64].astype(np.float32)
                                    * nom.astype(np.float32)[..., None]).astype(f16)
                    U[..., 65:65 + ext] = (ef.astype(np.float32)
                                           * nom.astype(np.float32)[..., None]).astype(f16)
                    U[..., 64] = nom
                for wl in range(SG):
                    S = (dr[:, :, wl * B:(wl + 1) * B, None]
                         == np.arange(128, dtype=f16)[None, None, None, :])
                    ps = np.zeros((M, 128), np.float32)
                    for q in range(NQ):
                        for j in range(B):
                            ch = wl * B + j
                            if attn:
                                lhsT = U[:, q, ch, :].astype(np.float32)
                            else:
                                lhsT = land[:, q, ch, col0:col0 + 64].astype(np.float32)
                            ps += lhsT.T @ S[:, q, j, :].astype(np.float32)
                    acc[:, sg * SG + wl, :] = ps.astype(f16)
            acc_all[tname] = acc
        # finalize
        w2ss = im["w2a_ss"][64:75].astype(np.float32)
        w2os = im["w2a_os"][64:67].astype(np.float32)
        wfin = im["wfin"].astype(np.float32)
        bfin = im["bfin"].astype(np.float32)
        for w in range(NWIN):
            n0 = w * 128
            a_ss = acc_all["ss"][:, w, :].astype(np.float32)
            a_os = acc_all["os"][:, w, :].astype(np.float32)
            num_ss = a_ss[0:64] + w2ss.T @ a_ss[64:75]
            num_os = a_os[0:64] + w2os.T @ a_os[64:67]
            den_ss = np.maximum(a_ss[64], 1e-20)
            den_os = np.maximum(a_os[64], 1e-20)
            z = (num_ss.astype(f16).astype(np.float32) / den_ss[None, :]
                 + num_os.astype(f16).astype(np.float32) / den_os[None, :])
            out[0, n0:n0 + 128, :] = z.T.astype(f16)
            fsb = im["feat"][1, :, n0:n0 + 128].astype(np.float32)
            hself = wfin[:, 3, :].T @ fsb + bfin[:, 1, :]
            r_fw = np.maximum(acc_all["fw"][0:64, w, :].astype(np.float32), 0)
            r_self = np.maximum(hself, 0).astype(f16).astype(np.float32)
            r_bw = np.maximum(acc_all["bw"][0:64, w, :].astype(np.float32), 0)
            x = (wfin[:, 0, :].T @ r_fw.astype(f16).astype(np.float32)
                 + wfin[:, 1, :].T @ r_self
                 + wfin[:, 2, :].T @ r_bw.astype(f16).astype(np.float32)
                 + bfin[:, 0, :])
            out[1, n0:n0 + 128, :] = x.T.astype(f16)
        results[c]["out"] = out
    return results


def _assemble(results):
    z = np.concatenate(
        [results[c]["out"][0, :SHARD, :].astype(np.float32) for c in range(NC)],
        axis=0)
    x = np.concatenate(
        [results[c]["out"][1, :SHARD, :].astype(np.float32) for c in range(NC)],
        axis=0)
    return z, x


def kernel_sim(**inputs):
    inp = {k: np.asarray(v) for k, v in inputs.items()}
    in_maps = _pack(inp)
    return _assemble(_simulate(in_maps))


def kernel(**inputs):
    global _PROGRAM, LAST_DEVICE_WALL_NS
    from concourse.bass_utils import run_bass_kernel_spmd
    inp = {k: np.asarray(v) for k, v in inputs.items()}
    in_maps = _pack(inp)
    if _PROGRAM is None:
        _PROGRAM = _build_program()
    import time as _time
    _t0 = _time.time()
    res = run_bass_kernel_spmd(_PROGRAM, in_maps, list(range(NC)))
    LAST_DEVICE_WALL_NS = (_time.time() - _t0) * 1e9
    return _assemble(res.results)


# revision 3
# speedup vs baseline: 1.9293x; 1.1230x over previous
"""Trainium2 Bass kernel for nn_AttnConvLayer (GNN message passing), v2.

Wire-optimized vs v1: inputs ~97MB total (vs ~740MB), outputs 25.6MB
(vs 108MB), everything else moved on-device:
  - s_feat/o_feat shipped fp16, feature-major, sharded 1/8 per core;
    AllGather on device; the three 256B-row gather tables (t_s =
    [m_ss|qm_ss], t_o = [m_os|qm_os], t_x = [t_in|t_out]) are built on
    device with PE matmuls + PE transposes.
  - Tables padded per-core-segment to 12544 rows so quadrant-local
    gather indices fit int16 (4 quadrants x 25088 rows).
  - Edge shards: dst-sharded per core, 128-node dst windows, budget
    512 slots per (window, quadrant) -> overflow probability ~1e-17.
  - idx shipped compact [16, .] and replicated to 128 partitions on
    device; dr/c/ef fp16.
  - Full finalize on device: per-dst softmax normalize + W2 fold for z,
    relu/Wo combine + h_self for x; output [2, 12544, 64] fp16/core.
"""

import sys
sys.path.insert(0, '/opt/trn_rl_repo')
import numpy as np

N_S = 100000
N_O = 100000
D = 64
NC = 8
SHARD = N_S // NC          # 12500
WIN = 128
NWIN = 100                 # padded window count (12800 nodes/core)
NODES = NWIN * WIN         # 12800
FINWIN = 98                # windows with real nodes (<= 12544 covers 12500)
SUP = 512                  # supergroup = 4 windows, one-hot width
SGW = SUP // WIN           # 4 windows per supergroup
NSG = NWIN // SGW          # 25 supergroups
CHUNKS_SG = 12             # chunks per (supergroup, quadrant)
SLOTS_W = CHUNKS_SG * 128  # 1536 slots per (supergroup, quadrant)
TOK = SLOTS_W              # tokens per (sg, q) gather
NQ = 4
QROWS = 2 * NODES          # 25600 table rows per quadrant
TROWS = NC * NODES         # 102400
HALF_SGS = (13, 12)        # supergroups per finalize half
HALF_W = (52, 48)          # windows per finalize half

# name, ext, attn, table idx, col0, M
TYPES = [
    ("ss", 10, True, 0, 0, 75),
    ("os", 2, True, 1, 0, 67),
    ("fw", 0, False, 2, 0, 64),
    ("bw", 0, False, 2, 64, 64),
]

_PROGRAM = None
_RUNNER = None
LAST_DEVICE_WALL_NS = None


def _blob_spec():
    """Ordered (name, shape) of all per-core inputs packed into one int16
    blob. All entries are 2-byte (f16 viewed as i16, or i16)."""
    spec = [
        ("feat", (2, D, NODES)),
        ("wc", (D, 3, 128)),
        ("biasx", (128, 1)),
        ("w2a_ss", (75, 64)),
        ("w2a_os", (67, 64)),
        ("wfin", (D, 4, 64)),
        ("bfin", (D, 2, 1)),
    ]
    for t, ext, attn, _, _, _ in TYPES:
        spec.append((f"idx_{t}", (NSG, 16, NQ, TOK // 16)))
        spec.append((f"dr_{t}", (NSG, 128, NQ, CHUNKS_SG)))
        if attn:
            spec.append((f"c_{t}", (NSG, 128, NQ, CHUNKS_SG)))
            spec.append((f"ef_{t}", (NSG, 128, NQ, CHUNKS_SG, ext)))
    offs = {}
    off = 0
    for name, shape in spec:
        n = int(np.prod(shape))
        offs[name] = (off, shape)
        off += n
    return offs, off


# ---------------------------------------------------------------- host pack

def _pack(inp):
    f16 = np.float16
    s_feat = inp["s_feat"].astype(np.float32)
    o_feat = inp["o_feat"].astype(np.float32)
    Wss_w, Wss_b = inp["Wss_w"].astype(np.float32), inp["Wss_b"].astype(np.float32)
    Wos_w, Wos_b = inp["Wos_w"].astype(np.float32), inp["Wos_b"].astype(np.float32)
    Ws_w, Ws_b = inp["Ws_w"].astype(np.float32), inp["Ws_b"].astype(np.float32)
    attn_w, attn_b = inp["attn_w"].astype(np.float32), inp["attn_b"].astype(np.float32)
    Win_w, Win_b = inp["Win_w"].astype(np.float32), inp["Win_b"].astype(np.float32)
    Wself_w, Wself_b = inp["Wself_w"].astype(np.float32), inp["Wself_b"].astype(np.float32)
    Wout_w, Wout_b = inp["Wout_w"].astype(np.float32), inp["Wout_b"].astype(np.float32)
    Wo_w, Wo_b = inp["Wo_w"].astype(np.float32), inp["Wo_b"].astype(np.float32)

    aw1 = attn_w[:D, 0]
    aw2 = attn_w[D:, 0]
    W2ss = Wss_w[D:]     # [10, 64]
    W2os = Wos_w[D:]     # [2, 64]

    # ---- feature shards, feature-major fp16, padded to NODES cols ----
    sT = np.ascontiguousarray(s_feat.T.astype(f16))    # [64, N_S]
    oT = np.ascontiguousarray(o_feat.T.astype(f16))
    feat = np.zeros((NC, 2, D, NODES), f16)
    feat[:, 0, :, :SHARD] = sT.reshape(D, NC, SHARD).transpose(1, 0, 2)
    feat[:, 1, :, :SHARD] = oT.reshape(D, NC, SHARD).transpose(1, 0, 2)

    # ---- small weights ----
    wc = np.zeros((D, 3, 128), f16)
    wc[:, 0, 0:64] = Wss_w[:D]
    wc[:, 0, 64] = Wss_w[:D] @ aw1
    wc[:, 1, 0:64] = Wos_w[:D]
    wc[:, 1, 64] = Wos_w[:D] @ aw1
    wc[:, 2, 0:64] = Win_w
    wc[:, 2, 64:128] = Wout_w
    biasx = np.concatenate([Win_b, Wout_b]).astype(f16)[:, None]  # [128,1]

    w2a_ss = np.zeros((75, 64), f16)
    w2a_ss[64] = Wss_b
    w2a_ss[65:75] = W2ss
    w2a_os = np.zeros((67, 64), f16)
    w2a_os[64] = Wos_b
    w2a_os[65:67] = W2os

    wfin = np.zeros((D, 4, 64), f16)
    wfin[:, 0, :] = Wo_w[0:64]      # h_in path
    wfin[:, 1, :] = Wo_w[64:128]    # h_self path
    wfin[:, 2, :] = Wo_w[128:192]   # h_out path
    wfin[:, 3, :] = Wself_w
    bfin = np.zeros((D, 2, 1), f16)
    bfin[:, 0, 0] = Wo_b
    bfin[:, 1, 0] = Wself_b

    # ---- attention edge constants (host) ----
    h_s = s_feat @ Ws_w + Ws_b
    a2 = h_s @ aw2
    ef_ss = inp["efeat_ss"].astype(np.float32)
    ef_os = inp["efeat_os"].astype(np.float32)
    c_ss = ef_ss @ (W2ss @ aw1) + (Wss_b @ aw1 + attn_b[0]) \
        + a2[np.asarray(inp["ss_dst"]).astype(np.int64)]
    c_os = ef_os @ (W2os @ aw1) + (Wos_b @ aw1 + attn_b[0]) \
        + a2[np.asarray(inp["os_dst"]).astype(np.int64)]

    edge_cfg = {
        "ss": (inp["ss_src"], inp["ss_dst"], c_ss, ef_ss, 10),
        "os": (inp["os_src"], inp["os_dst"], c_os, ef_os, 2),
        "fw": (inp["fwd_src"], inp["fwd_dst"], None, None, 0),
        "bw": (inp["bwd_src"], inp["bwd_dst"], None, None, 0),
    }

    in_maps = [dict() for _ in range(NC)]
    for c in range(NC):
        in_maps[c]["feat"] = feat[c]
        in_maps[c]["wc"] = wc
        in_maps[c]["biasx"] = biasx
        in_maps[c]["w2a_ss"] = w2a_ss
        in_maps[c]["w2a_os"] = w2a_os
        in_maps[c]["wfin"] = wfin
        in_maps[c]["bfin"] = bfin

    for t, (src, dst, c_e, ef, ext) in edge_cfg.items():
        src = np.asarray(src).astype(np.int64)
        dst = np.asarray(dst).astype(np.int64)
        E = src.shape[0]
        core = dst // SHARD
        ldst = dst - core * SHARD
        sg = ldst // SUP
        drel = (ldst - sg * SUP).astype(f16)
        r = (src // SHARD) * NODES + (src - (src // SHARD) * SHARD)
        q = r // QROWS
        lsrc = (r - q * QROWS).astype(np.int16)

        gid = ((core * NSG + sg) * NQ + q)
        NG = NC * NSG * NQ
        order = np.argsort(gid, kind="stable")
        cnt = np.bincount(gid, minlength=NG)
        starts = np.zeros(NG + 1, np.int64)
        np.cumsum(cnt, out=starts[1:])
        rank = np.empty(E, np.int64)
        rank[order] = np.arange(E) - starts[gid[order]]
        if not (rank < SLOTS_W).all():
            # ~1e-17 probability; drop excess edges rather than crash
            keep = rank < SLOTS_W
            src, dst, core, sg, q, lsrc, drel, rank = (
                a[keep] for a in (src, dst, core, sg, q, lsrc, drel, rank))
            if c_e is not None:
                c_e, ef = c_e[keep], ef[keep]
        tok = rank

        idx_a = np.zeros((NC, NSG, NQ, TOK), np.int16)
        dr_a = np.full((NC, NSG, NQ, TOK), -1.0, f16)
        idx_a[core, sg, q, tok] = lsrc
        dr_a[core, sg, q, tok] = drel
        # device layouts
        idx_w = np.ascontiguousarray(
            idx_a.reshape(NC, NSG, NQ, TOK // 16, 16).transpose(0, 1, 4, 2, 3))
        # [NC, NSG, 16, NQ, TOK//16]
        dr_w = np.ascontiguousarray(
            dr_a.reshape(NC, NSG, NQ, CHUNKS_SG, 128).transpose(0, 1, 4, 2, 3))
        # [NC, NSG, 128, NQ, CHUNKS_SG]
        for c in range(NC):
            in_maps[c][f"idx_{t}"] = idx_w[c]
            in_maps[c][f"dr_{t}"] = dr_w[c]
        if c_e is not None:
            cc_a = np.zeros((NC, NSG, NQ, TOK), f16)
            cc_a[core, sg, q, tok] = c_e.astype(f16)
            cc_w = np.ascontiguousarray(
                cc_a.reshape(NC, NSG, NQ, CHUNKS_SG, 128).transpose(0, 1, 4, 2, 3))
            ef_a = np.zeros((NC, NSG, NQ, TOK, ext), f16)
            ef_a[core, sg, q, tok] = ef.astype(f16)
            ef_w = np.ascontiguousarray(
                ef_a.reshape(NC, NSG, NQ, CHUNKS_SG, 128, ext)
                .transpose(0, 1, 4, 2, 3, 5))
            for c in range(NC):
                in_maps[c][f"c_{t}"] = cc_w[c]
                in_maps[c][f"ef_{t}"] = ef_w[c]
    return in_maps


# ---------------------------------------------------------------- bass build

def _build_program():
    from concourse import bass, bacc, mybir
    import concourse.tile as tile

    F16 = mybir.dt.float16
    F32 = mybir.dt.float32
    I16 = mybir.dt.int16
    AF = mybir.ActivationFunctionType
    OP = mybir.AluOpType

    nc = bacc.Bacc(None, target_bir_lowering=False, num_devices=NC,
                   dynamic_dma_scratch_size=2 ** 15)

    inp = {}
    inp["feat"] = nc.declare_dram_parameter("feat", [2, D, NODES], F16, isOutput=False)
    inp["wc"] = nc.declare_dram_parameter("wc", [D, 3, 128], F16, isOutput=False)
    inp["biasx"] = nc.declare_dram_parameter("biasx", [128, 1], F32, isOutput=False)
    inp["w2a_ss"] = nc.declare_dram_parameter("w2a_ss", [75, 64], F16, isOutput=False)
    inp["w2a_os"] = nc.declare_dram_parameter("w2a_os", [67, 64], F16, isOutput=False)
    inp["wfin"] = nc.declare_dram_parameter("wfin", [D, 4, 64], F16, isOutput=False)
    inp["bfin"] = nc.declare_dram_parameter("bfin", [D, 2, 1], F32, isOutput=False)
    inp["iden"] = nc.declare_dram_parameter("iden", [128, 128], F16, isOutput=False)
    inp["iota"] = nc.declare_dram_parameter("iota", [128, 1, 128], F16, isOutput=False)
    for t, ext, attn, _, _, _ in TYPES:
        inp[f"idx_{t}"] = nc.declare_dram_parameter(
            f"idx_{t}", [NSG, 16, NQ, TOK // 16], I16, isOutput=False)
        inp[f"dr_{t}"] = nc.declare_dram_parameter(
            f"dr_{t}", [NSG, 128, NQ, CHUNKS_SG], F16, isOutput=False)
        if attn:
            inp[f"c_{t}"] = nc.declare_dram_parameter(
                f"c_{t}", [NSG, 128, NQ, CHUNKS_SG], F16, isOutput=False)
            inp[f"ef_{t}"] = nc.declare_dram_parameter(
                f"ef_{t}", [NSG, 128, NQ, CHUNKS_SG, ext], F16, isOutput=False)
    out = nc.declare_dram_parameter("out", [2, NODES, D], F16, isOutput=True)

    with tile.TileContext(nc) as tc:
        with tc.tile_pool(name="dram", bufs=1, space="DRAM") as dram:
            bounce = dram.tile([2, D, NODES], F16)
            featg = dram.tile([NC, 2, D, NODES], F16, addr_space="Shared")
            tbl = dram.tile([3, TROWS, 128], F16)

            nc.sync.dma_start(out=bounce[:, :, :], in_=inp["feat"][:, :, :])
            nc.gpsimd.collective_compute(
                "AllGather", OP.bypass,
                replica_groups=[list(range(NC))],
                ins=[bounce[:, :, :].opt()],
                outs=[featg[:, :, :, :].opt()],
            )

            # ---------------- phase A: build tables ----------------
            with (
                tc.tile_pool(name="tconst", bufs=1) as tcp,
                tc.tile_pool(name="tbuild", bufs=3) as tp,
                tc.tile_pool(name="tpsum", bufs=2, space="PSUM") as tpp,
            ):
                wc_sb = tcp.tile([D, 3, 128], F16)
                nc.sync.dma_start(out=wc_sb[:, :, :], in_=inp["wc"][:, :, :])
                bx_sb = tcp.tile([128, 1], F16)
                nc.sync.dma_start(out=bx_sb[:, :], in_=inp["biasx"][:, :])
                idA_sb = tcp.tile([128, 128], F16)
                nc.sync.dma_start(out=idA_sb[:, :], in_=inp["iden"][:, :])

                for c8 in range(NC):
                    for t in range(3):
                        srcf = 0 if t == 0 else 1
                        for j0 in range(0, NODES, 512):
                            W = min(512, NODES - j0)
                            KT = W // 128
                            rsb = tp.tile([D, 512], F16, tag="rsb")
                            nc.sync.dma_start(
                                out=rsb[:, :W], in_=featg[c8, srcf, :, j0:j0 + W])
                            ps = tpp.tile([128, 512], F32, tag="psA")
                            nc.tensor.matmul(ps[:, :W], wc_sb[:, t, :], rsb[:, :W],
                                             start=True, stop=True)
                            csb = tp.tile([128, 512], F16, tag="csb")
                            if t == 2:
                                nc.scalar.activation(csb[:, :W], ps[:, :W],
                                                     AF.Identity, bias=bx_sb[:, :])
                            else:
                                nc.scalar.activation(csb[:, :W], ps[:, :W], AF.Copy)
                            ps2 = tpp.tile([128, 4, 128], F32, tag="psA2")
                            for k in range(KT):
                                nc.tensor.matmul(
                                    ps2[:, k, :], csb[:, k * 128:(k + 1) * 128],
                                    idA_sb[:, :], start=True, stop=True)
                            osb = tp.tile([128, 4, 128], F16, tag="osb")
                            if (j0 // 512) % 2 == 0:
                                nc.vector.tensor_copy(out=osb[:, :KT, :],
                                                      in_=ps2[:, :KT, :])
                            else:
                                nc.scalar.activation(osb[:, :KT, :], ps2[:, :KT, :],
                                                     AF.Copy)
                            base = c8 * NODES + j0
                            nc.sync.dma_start(
                                out=tbl[t, base:base + W, :].rearrange(
                                    "(k p) f -> p k f", p=128),
                                in_=osb[:, :KT, :])

            # ---------------- phase B: edges + finalize ----------------
            with (
                tc.tile_pool(name="const", bufs=1) as cp,
                tc.tile_pool(name="acc", bufs=1) as ap_,
                tc.tile_pool(name="work", bufs=2) as wp,
                tc.tile_pool(name="small", bufs=3) as sp,
                tc.tile_pool(name="eps", bufs=3, space="PSUM") as epp,
                tc.tile_pool(name="fin", bufs=1, space="PSUM") as fpp,
            ):
                iden_sb = cp.tile([128, 128], F16)
                nc.sync.dma_start(out=iden_sb[:, :], in_=inp["iden"][:, :])
                iota_sb = cp.tile([128, 1, 128], F16)
                nc.sync.dma_start(out=iota_sb[:, :, :], in_=inp["iota"][:, :, :])
                w2ss_sb = cp.tile([75, 64], F16)
                nc.sync.dma_start(out=w2ss_sb[:, :], in_=inp["w2a_ss"][:, :])
                w2os_sb = cp.tile([67, 64], F16)
                nc.sync.dma_start(out=w2os_sb[:, :], in_=inp["w2a_os"][:, :])
                wfin_sb = cp.tile([D, 4, 64], F16)
                nc.sync.dma_start(out=wfin_sb[:, :, :], in_=inp["wfin"][:, :, :])
                bfin_sb = cp.tile([D, 2, 1], F16)
                nc.sync.dma_start(out=bfin_sb[:, :, :], in_=inp["bfin"][:, :, :])

                for half in range(2):
                    accs = {}
                    for (tname, ext, attn, tq, col0, M) in TYPES:
                        acc = ap_.tile([75, HALF, 128], F16, tag=f"acc_{tname}")
                        accs[tname] = acc
                        for sgl in range(NSG // 2):
                            sg = half * (NSG // 2) + sgl
                            idx_sb = wp.tile([128, NQ, TOK // 16], I16, tag="idx")
                            for k in range(8):
                                nc.sync.dma_start(
                                    out=idx_sb[16 * k:16 * (k + 1), :, :],
                                    in_=inp[f"idx_{tname}"][sg, :, :, :])
                            dr_sb = wp.tile([128, NQ, CHUNKS_SG], F16, tag="dr")
                            nc.sync.dma_start(out=dr_sb[:, :, :],
                                              in_=inp[f"dr_{tname}"][sg, :, :, :])
                            land = wp.tile([128, NQ, CHUNKS_SG, 128], F16, tag="land")
                            for q in range(NQ):
                                nc.gpsimd.dma_gather(
                                    out_ap=land[:, q, :, :],
                                    in_ap=tbl[tq, q * QROWS:(q + 1) * QROWS, :],
                                    idxs_ap=idx_sb[:, q, :],
                                    num_idxs=TOK,
                                    num_idxs_reg=TOK,
                                    elem_size=128,
                                    single_packet=False,
                                )
                            if attn:
                                c_sb = wp.tile([128, NQ, CHUNKS_SG], F16, tag="cc")
                                nc.sync.dma_start(out=c_sb[:, :, :],
                                                  in_=inp[f"c_{tname}"][sg, :, :, :])
                                ef_sb = wp.tile([128, NQ, CHUNKS_SG, 10], F16,
                                                tag="ef")
                                nc.sync.dma_start(
                                    out=ef_sb[:, :, :, :ext],
                                    in_=inp[f"ef_{tname}"][sg, :, :, :, :])
                                sv = wp.tile([128, NQ, CHUNKS_SG, 1], F32, tag="sv")
                                nc.vector.tensor_tensor(
                                    out=sv[:, :, :, :], in0=land[:, :, :, 64:65],
                                    in1=c_sb[:, :, :].unsqueeze(3), op=OP.add)
                                nc.scalar.activation(sv[:, :, :, :], sv[:, :, :, :],
                                                     AF.Lrelu, alpha=0.01)
                                nom = wp.tile([128, NQ, CHUNKS_SG, 1], F16, tag="nom")
                                nc.scalar.activation(nom[:, :, :, :], sv[:, :, :, :],
                                                     AF.Exp)
                                U = wp.tile([128, NQ, CHUNKS_SG, 75], F16, tag="U")
                                nc.vector.tensor_tensor(
                                    out=U[:, :, :, 0:64], in0=land[:, :, :, 0:64],
                                    in1=nom[:, :, :, :].to_broadcast(
                                        [128, NQ, CHUNKS_SG, 64]),
                                    op=OP.mult)
                                nc.vector.tensor_tensor(
                                    out=U[:, :, :, 65:65 + ext],
                                    in0=ef_sb[:, :, :, :ext],
                                    in1=nom[:, :, :, :].to_broadcast(
                                        [128, NQ, CHUNKS_SG, ext]),
                                    op=OP.mult)
                                nc.scalar.activation(
                                    U[:, :, :, 64:65], nom[:, :, :, :], AF.Copy)
                            for wl in range(SG):
                                S = sp.tile([128, NQ, B, 128], F16, tag="S")
                                nc.vector.tensor_tensor(
                                    out=S[:, :, :, :],
                                    in0=dr_sb[:, :, wl * B:(wl + 1) * B]
                                    .unsqueeze(3).to_broadcast([128, NQ, B, 128]),
                                    in1=iota_sb[:, 0:1, :].unsqueeze(1)
                                    .to_broadcast([128, NQ, B, 128]),
                                    op=OP.is_equal)
                                ps = epp.tile([75, 128], F32, tag="eps")
                                for q in range(NQ):
                                    for j in range(B):
                                        ch = wl * B + j
                                        if attn:
                                            lhsT = U[:, q, ch, 0:M]
                                        else:
                                            lhsT = land[:, q, ch, col0:col0 + 64]
                                        nc.tensor.matmul(
                                            ps[0:M, :], lhsT, S[:, q, j, :],
                                            start=(q == 0 and j == 0),
                                            stop=(q == NQ - 1 and j == B - 1))
                                wloc = sgl * SG + wl
                                nc.vector.tensor_copy(out=acc[0:M, wloc, :],
                                                      in_=ps[0:M, :])
                    # ---- finalize this half ----
                    for wloc in range(HALF):
                        n0 = (half * HALF + wloc) * 128
                        a_ss, a_os = accs["ss"], accs["os"]
                        a_fw, a_bw = accs["fw"], accs["bw"]
                        nm = fpp.tile([64, 2, 128], F32, tag="nm")
                        nc.tensor.matmul(nm[:, 0, :], iden_sb[0:64, 0:64],
                                         a_ss[0:64, wloc, :], start=True, stop=False)
                        nc.tensor.matmul(nm[:, 0, :], w2ss_sb[64:75, :],
                                         a_ss[64:75, wloc, :], start=False, stop=True)
                        nc.tensor.matmul(nm[:, 1, :], iden_sb[0:64, 0:64],
                                         a_os[0:64, wloc, :], start=True, stop=False)
                        nc.tensor.matmul(nm[:, 1, :], w2os_sb[64:67, :],
                                         a_os[64:67, wloc, :], start=False, stop=True)
                        dT = fpp.tile([128, 2, 1], F32, tag="dT")
                        nc.tensor.matmul(dT[:, 0, :], a_ss[64:65, wloc, :],
                                         iden_sb[64:65, 64:65], start=True, stop=True)
                        nc.tensor.matmul(dT[:, 1, :], a_os[64:65, wloc, :],
                                         iden_sb[64:65, 64:65], start=True, stop=True)
                        dmx = sp.tile([128, 2, 1], F32, tag="dmx")
                        nc.vector.tensor_scalar_max(dmx[:, :, :], dT[:, :, :], 1e-20)
                        rec = sp.tile([128, 2, 1], F32, tag="rec")
                        nc.vector.reciprocal(rec[:, :, :], dmx[:, :, :])
                        nsb = sp.tile([64, 2, 128], F16, tag="nsb")
                        nc.scalar.activation(nsb[:, :, :], nm[:, :, :], AF.Copy)
                        zT = fpp.tile([128, 2, 64], F32, tag="zT")
                        nc.tensor.matmul(zT[:, 0, :], nsb[:, 0, :],
                                         iden_sb[0:64, 0:64], start=True, stop=True)
                        nc.tensor.matmul(zT[:, 1, :], nsb[:, 1, :],
                                         iden_sb[0:64, 0:64], start=True, stop=True)
                        zp = sp.tile([128, 2, 64], F32, tag="zp")
                        nc.vector.tensor_scalar(
                            out=zp[:, 0, :], in0=zT[:, 0, :],
                            scalar1=rec[:, 0, :], scalar2=None, op0=OP.mult)
                        nc.vector.tensor_scalar(
                            out=zp[:, 1, :], in0=zT[:, 1, :],
                            scalar1=rec[:, 1, :], scalar2=None, op0=OP.mult)
                        zo = sp.tile([128, 64], F16, tag="zo")
                        nc.vector.tensor_tensor(out=zo[:, :], in0=zp[:, 0, :],
                                                in1=zp[:, 1, :], op=OP.add)
                        nc.sync.dma_start(out=out[0, n0:n0 + 128, :], in_=zo[:, :])
                        # x path
                        fsb = sp.tile([64, 128], F16, tag="fsb")
                        nc.sync.dma_start(out=fsb[:, :],
                                          in_=inp["feat"][1, :, n0:n0 + 128])
                        hx = fpp.tile([64, 2, 128], F32, tag="hx")
                        nc.tensor.matmul(hx[:, 0, :], wfin_sb[:, 3, :], fsb[:, :],
                                         start=True, stop=True)
                        rl = sp.tile([64, 3, 128], F16, tag="rl")
                        nc.scalar.activation(rl[:, 0, :], a_fw[0:64, wloc, :],
                                             AF.Relu)
                        nc.scalar.activation(rl[:, 1, :], hx[:, 0, :], AF.Relu,
                                             bias=bfin_sb[:, 1, :])
                        nc.scalar.activation(rl[:, 2, :], a_bw[0:64, wloc, :],
                                             AF.Relu)
                        nc.tensor.matmul(hx[:, 1, :], wfin_sb[:, 0, :], rl[:, 0, :],
                                         start=True, stop=False)
                        nc.tensor.matmul(hx[:, 1, :], wfin_sb[:, 1, :], rl[:, 1, :],
                                         start=False, stop=False)
                        nc.tensor.matmul(hx[:, 1, :], wfin_sb[:, 2, :], rl[:, 2, :],
                                         start=False, stop=True)
                        xsb = sp.tile([64, 128], F16, tag="xsb")
                        nc.scalar.activation(xsb[:, :], hx[:, 1, :], AF.Identity,
                                             bias=bfin_sb[:, 0, :])
                        xT = fpp.tile([128, 64], F32, tag="xT")
                        nc.tensor.matmul(xT[:, :], xsb[:, :], iden_sb[0:64, 0:64],
                                         start=True, stop=True)
                        xo = sp.tile([128, 64], F16, tag="xo")
                        nc.scalar.activation(xo[:, :], xT[:, :], AF.Copy)
                        nc.sync.dma_start(out=out[1, n0:n0 + 128, :], in_=xo[:, :])

    nc.finalize()
    return nc


# ---------------------------------------------------------------- numpy sim

def _simulate(in_maps):
    """Numpy emulation of the device program (fp16 rounding where it
    matters) — validates packing + math without compiling."""
    f16 = np.float16
    results = []
    for c in range(NC):
        results.append({})
    # AllGather
    featg = np.stack([in_maps[c]["feat"] for c in range(NC)])  # [NC,2,64,NODES]
    wc = in_maps[0]["wc"].astype(np.float32)
    biasx = in_maps[0]["biasx"].astype(np.float32)
    # tables (same on all cores)
    tbl = np.zeros((3, TROWS, 128), f16)
    for t in range(3):
        srcf = 0 if t == 0 else 1
        ft = featg[:, srcf].transpose(0, 2, 1).reshape(TROWS, D)  # [TROWS, 64]
        m = ft.astype(np.float32) @ wc[:, t, :]
        if t == 2:
            m = m + biasx[:, 0][None, :]
        tbl[t] = m.astype(f16)

    for c in range(NC):
        im = in_maps[c]
        out = np.zeros((2, NODES, D), f16)
        acc_all = {}
        for (tname, ext, attn, tq, col0, M) in TYPES:
            acc = np.zeros((M, NWIN, 128), f16)
            for sg in range(NSG):
                idx = im[f"idx_{tname}"][sg]      # [16, NQ, 224]
                dr = im[f"dr_{tname}"][sg]        # [128, NQ, 28]
                # reconstruct tokens: token t at [t%16, q, t//16]
                toks = idx.transpose(1, 2, 0).reshape(NQ, TOK)  # [NQ, TOK]
                land = np.zeros((128, NQ, CHUNKS_SG, 128), f16)
                for q in range(NQ):
                    g = tbl[tq, q * QROWS + toks[q].astype(np.int64), :]
                    land[:, q, :, :] = g.reshape(CHUNKS_SG, 128, 128).transpose(1, 0, 2)
                if attn:
                    cc = im[f"c_{tname}"][sg]     # [128, NQ, 28]
                    ef = im[f"ef_{tname}"][sg]    # [128, NQ, 28, ext]
                    sv = land[:, :, :, 64].astype(np.float32) + cc.astype(np.float32)
                    sv = np.where(sv > 0, sv, 0.01 * sv)
                    nom = np.exp(sv).astype(f16)
                    U = np.zeros((128, NQ, CHUNKS_SG, M), f16)
                    U[..., 0:64] = (land[..., 0:64].astype(np.float32)
                                    * nom.astype(np.float32)[..., None]).astype(f16)
                    U[..., 65:65 + ext] = (ef.astype(np.float32)
                                           * nom.astype(np.float32)[..., None]).astype(f16)
                    U[..., 64] = nom
                for wl in range(SG):
                    S = (dr[:, :, wl * B:(wl + 1) * B, None]
                         == np.arange(128, dtype=f16)[None, None, None, :])
                    ps = np.zeros((M, 128), np.float32)
                    for q in range(NQ):
                        for j in range(B):
                            ch = wl * B + j
                            if attn:
                                lhsT = U[:, q, ch, :].astype(np.float32)
                            else:
                                lhsT = land[:, q, ch, col0:col0 + 64].astype(np.float32)
                            ps += lhsT.T @ S[:, q, j, :].astype(np.float32)
                    acc[:, sg * SG + wl, :] = ps.astype(f16)
            acc_all[tname] = acc
        # finalize
        w2ss = im["w2a_ss"][64:75].astype(np.float32)
        w2os = im["w2a_os"][64:67].astype(np.float32)
        wfin = im["wfin"].astype(np.float32)
        bfin = im["bfin"].astype(np.float32)
        for w in range(NWIN):
            n0 = w * 128
            a_ss = acc_all["ss"][:, w, :].astype(np.float32)
            a_os = acc_all["os"][:, w, :].astype(np.float32)
            num_ss = a_ss[0:64] + w2ss.T @ a_ss[64:75]
            num_os = a_os[0:64] + w2os.T @ a_os[64:67]
            den_ss = np.maximum(a_ss[64], 1e-20)
            den_os = np.maximum(a_os[64], 1e-20)
            z = (num_ss.astype(f16).astype(np.float32) / den_ss[None, :]
                 + num_os.astype(f16).astype(np.float32) / den_os[None, :])
            out[0, n0:n0 + 128, :] = z.T.astype(f16)
            fsb = im["feat"][1, :, n0:n0 + 128].astype(np.float32)
            hself = wfin[:, 3, :].T @ fsb + bfin[:, 1, :]
            r_fw = np.maximum(acc_all["fw"][0:64, w, :].astype(np.float32), 0)
            r_self = np.maximum(hself, 0).astype(f16).astype(np.float32)
            r_bw = np.maximum(acc_all["bw"][0:64, w, :].astype(np.float32), 0)
            x = (wfin[:, 0, :].T @ r_fw.astype(f16).astype(np.float32)
                 + wfin[:, 1, :].T @ r_self
                 + wfin[:, 2, :].T @ r_bw.astype(f16).astype(np.float32)
                 + bfin[:, 0, :])
            out[1, n0:n0 + 128, :] = x.T.astype(f16)
        results[c]["out"] = out
    return results


def _assemble(results):
    z = np.concatenate(
        [results[c]["out"][0, :SHARD, :].astype(np.float32) for c in range(NC)],
        axis=0)
    x = np.concatenate(
        [results[c]["out"][1, :SHARD, :].astype(np.float32) for c in range(NC)],
        axis=0)
    return z, x


def kernel_sim(**inputs):
    inp = {k: np.asarray(v) for k, v in inputs.items()}
    in_maps = _pack(inp)
    return _assemble(_simulate(in_maps))


def _blobify(in_maps):
    offs, total = _blob_spec()
    blobs = np.empty((NC, total), np.int16)
    for c in range(NC):
        b = blobs[c]
        for name, (off, shape) in offs.items():
            n = int(np.prod(shape))
            b[off:off + n] = in_maps[c][name].ravel().view(np.int16)
    return blobs


def _get_runner():
    global _PROGRAM, _RUNNER
    if _RUNNER is not None:
        return _RUNNER
    import jax, jax.numpy as jnp
    from jax.sharding import Mesh, PartitionSpec, NamedSharding
    from jax.experimental.shard_map import shard_map
    from concourse import mybir
    from concourse.bass2jax import (_bass_exec_p, install_neuronx_cc_hook,
                                    partition_id_tensor)
    if _PROGRAM is None:
        _PROGRAM = _build_program()
    nc_ = _PROGRAM
    install_neuronx_cc_hook()
    partition_name = (nc_.partition_id_tensor.name
                      if nc_.partition_id_tensor else None)
    in_names, out_names, out_avals, zero_specs = [], [], [], []
    for alloc in nc_.m.functions[0].allocations:
        if not isinstance(alloc, mybir.MemoryLocationSet):
            continue
        if alloc.kind not in ("ExternalInput", "ExternalOutput"):
            continue
        name = alloc.memorylocations[0].name
        if alloc.kind == "ExternalInput":
            if name != partition_name:
                in_names.append(name)
        else:
            shape = tuple(alloc.tensor_shape)
            dtype = mybir.dt.np(alloc.dtype)
            out_names.append(name)
            out_avals.append(jax.core.ShapedArray(shape, dtype))
            zero_specs.append((shape, dtype))
    n_params = len(in_names)
    bind_names = tuple(in_names + out_names
                       + ([partition_name] if partition_name else []))
    donate = tuple(range(n_params, n_params + len(out_names)))

    def _body(*args):
        operands = list(args)
        if partition_name is not None:
            operands.append(partition_id_tensor())
        outs = _bass_exec_p.bind(
            *operands, out_avals=tuple(out_avals), in_names=bind_names,
            out_names=tuple(out_names), lowering_input_output_aliases=(),
            sim_require_finite=True, sim_require_nnan=True, nc=nc_)
        return tuple(outs)

    devs = jax.devices()[:NC]
    mesh = Mesh(np.asarray(devs), ("core",))
    ns = NamedSharding(mesh, PartitionSpec("core"))
    nin = n_params + len(out_names)
    sharded = jax.jit(
        shard_map(_body, mesh=mesh, in_specs=(PartitionSpec("core"),) * nin,
                  out_specs=(PartitionSpec("core"),) * len(out_names),
                  check_rep=False),
        donate_argnums=donate, keep_unused=True)
    zeros_fn = jax.jit(
        lambda: tuple(jnp.zeros((NC * s[0], *s[1:]), d) for s, d in zero_specs),
        out_shardings=(ns,) * len(zero_specs))
    _RUNNER = (sharded, zeros_fn, in_names, out_names, mesh, devs, ns)
    return _RUNNER


def kernel(**inputs):
    global LAST_DEVICE_WALL_NS
    import time as _time
    import jax
    from concurrent.futures import ThreadPoolExecutor
    inp = {k: np.asarray(v) for k, v in inputs.items()}
    in_maps = _pack(inp)
    blobs = _blobify(in_maps)
    sharded, zeros_fn, in_names, out_names, mesh, devs, ns = _get_runner()
    assert in_names == ["blob"], in_names
    _t0 = _time.time()

    def put_one(c):
        a = jax.device_put(blobs[c], devs[c])
        a.block_until_ready()
        return a

    with ThreadPoolExecutor(NC) as ex:
        bufs = list(ex.map(put_one, range(NC)))
    garr = jax.make_array_from_single_device_arrays(
        (NC * blobs.shape[1],), ns, bufs)
    zeros = zeros_fn()
    outs = sharded(garr, *zeros)
    jax.block_until_ready(outs)
    shards = sorted(outs[0].addressable_shards,
                    key=lambda sh: (sh.index[0].start or 0))
    with ThreadPoolExecutor(NC) as ex:
        parts = list(ex.map(lambda sh: np.asarray(sh.data), shards))
    LAST_DEVICE_WALL_NS = (_time.time() - _t0) * 1e9
    results = [{"out": parts[c]} for c in range(NC)]
    return _assemble(results)


# revision 4
# speedup vs baseline: 2.0809x; 1.0786x over previous
"""Trainium2 Bass kernel for nn_AttnConvLayer (GNN message passing), v2.

Wire-optimized vs v1: inputs ~97MB total (vs ~740MB), outputs 25.6MB
(vs 108MB), everything else moved on-device:
  - s_feat/o_feat shipped fp16, feature-major, sharded 1/8 per core;
    AllGather on device; the three 256B-row gather tables (t_s =
    [m_ss|qm_ss], t_o = [m_os|qm_os], t_x = [t_in|t_out]) are built on
    device with PE matmuls + PE transposes.
  - Tables padded per-core-segment to 12544 rows so quadrant-local
    gather indices fit int16 (4 quadrants x 25088 rows).
  - Edge shards: dst-sharded per core, 128-node dst windows, budget
    512 slots per (window, quadrant) -> overflow probability ~1e-17.
  - idx shipped compact [16, .] and replicated to 128 partitions on
    device; dr/c/ef fp16.
  - Full finalize on device: per-dst softmax normalize + W2 fold for z,
    relu/Wo combine + h_self for x; output [2, 12544, 64] fp16/core.
"""

import sys
sys.path.insert(0, '/opt/trn_rl_repo')
import numpy as np

N_S = 100000
N_O = 100000
D = 64
NC = 8
SHARD = N_S // NC          # 12500
WIN = 128
NWIN = 100                 # padded window count (12800 nodes/core)
NODES = NWIN * WIN         # 12800
FINWIN = 98                # windows with real nodes (<= 12544 covers 12500)
SUP = 512                  # supergroup = 4 windows, one-hot width
SGW = SUP // WIN           # 4 windows per supergroup
NSG = NWIN // SGW          # 25 supergroups
CHUNKS_SG = 12             # chunks per (supergroup, quadrant)
SLOTS_W = CHUNKS_SG * 128  # 1536 slots per (supergroup, quadrant)
TOK = SLOTS_W              # tokens per (sg, q) gather
NQ = 4
QROWS = 2 * NODES          # 25600 table rows per quadrant
TROWS = NC * NODES         # 102400
HALF_SGS = (13, 12)        # supergroups per finalize half
HALF_W = (52, 48)          # windows per finalize half

# name, ext, attn, table idx, col0, M
TYPES = [
    ("ss", 10, True, 0, 0, 75),
    ("os", 2, True, 1, 0, 67),
    ("fw", 0, False, 2, 0, 64),
    ("bw", 0, False, 2, 64, 64),
]

_PROGRAM = None
_RUNNER = None
LAST_DEVICE_WALL_NS = None
S_FEAT = 6.0 / 127.0       # int8 quantization scale for features
S_EF = 6.0 / 127.0         # int8 quantization scale for edge features


def _blob_spec():
    """Ordered (name, shape) of all per-core inputs packed into one int16
    blob. All entries are 2-byte (f16 viewed as i16, or i16)."""
    spec = [
        ("feat", (2, D, NODES), "f16"),
        ("wc", (D, 3, 128), "f16"),
        ("biasx", (128, 1), "f16"),
        ("w2a_ss", (75, 64), "f16"),
        ("w2a_os", (67, 64), "f16"),
        ("wfin", (D, 4, 64), "f16"),
        ("bfin", (D, 2, 1), "f16"),
    ]
    for t, ext, attn, _, _, _ in TYPES:
        spec.append((f"idx_{t}", (NSG, 16, NQ, TOK // 16), "i16"))
        spec.append((f"dr_{t}", (NSG, 128, NQ, CHUNKS_SG), "f16"))
        if attn:
            spec.append((f"c_{t}", (NSG, 128, NQ, CHUNKS_SG), "f16"))
            spec.append((f"ef_{t}", (NSG, 128, NQ, CHUNKS_SG, ext), "i8"))
    offs = {}
    off = 0
    for name, shape, dt in spec:
        n = int(np.prod(shape))
        assert dt != "i8" or n % 2 == 0
        n16 = n // 2 if dt == "i8" else n
        offs[name] = (off, shape, dt)
        off += n16
    return offs, off


# ---------------------------------------------------------------- host pack

def _pack(inp):
    f16 = np.float16
    s_feat = inp["s_feat"].astype(np.float32)
    o_feat = inp["o_feat"].astype(np.float32)
    Wss_w, Wss_b = inp["Wss_w"].astype(np.float32), inp["Wss_b"].astype(np.float32)
    Wos_w, Wos_b = inp["Wos_w"].astype(np.float32), inp["Wos_b"].astype(np.float32)
    Ws_w, Ws_b = inp["Ws_w"].astype(np.float32), inp["Ws_b"].astype(np.float32)
    attn_w, attn_b = inp["attn_w"].astype(np.float32), inp["attn_b"].astype(np.float32)
    Win_w, Win_b = inp["Win_w"].astype(np.float32), inp["Win_b"].astype(np.float32)
    Wself_w, Wself_b = inp["Wself_w"].astype(np.float32), inp["Wself_b"].astype(np.float32)
    Wout_w, Wout_b = inp["Wout_w"].astype(np.float32), inp["Wout_b"].astype(np.float32)
    Wo_w, Wo_b = inp["Wo_w"].astype(np.float32), inp["Wo_b"].astype(np.float32)

    aw1 = attn_w[:D, 0]
    aw2 = attn_w[D:, 0]
    W2ss = Wss_w[D:]     # [10, 64]
    W2os = Wos_w[D:]     # [2, 64]

    # ---- feature shards, feature-major fp16, padded to NODES cols ----
    sT = np.ascontiguousarray(s_feat.T.astype(f16))
    oT = np.ascontiguousarray(o_feat.T.astype(f16))
    feat = np.zeros((NC, 2, D, NODES), f16)
    feat[:, 0, :, :SHARD] = sT.reshape(D, NC, SHARD).transpose(1, 0, 2)
    feat[:, 1, :, :SHARD] = oT.reshape(D, NC, SHARD).transpose(1, 0, 2)

    # ---- small weights ----
    wc = np.zeros((D, 3, 128), f16)
    wc[:, 0, 0:64] = Wss_w[:D]
    wc[:, 0, 64] = Wss_w[:D] @ aw1
    wc[:, 1, 0:64] = Wos_w[:D]
    wc[:, 1, 64] = Wos_w[:D] @ aw1
    wc[:, 2, 0:64] = Win_w
    wc[:, 2, 64:128] = Wout_w
    biasx = np.concatenate([Win_b, Wout_b]).astype(f16)[:, None]  # [128,1]

    w2a_ss = np.zeros((75, 64), f16)
    w2a_ss[64] = Wss_b
    w2a_ss[65:75] = W2ss
    w2a_os = np.zeros((67, 64), f16)
    w2a_os[64] = Wos_b
    w2a_os[65:67] = W2os

    wfin = np.zeros((D, 4, 64), f16)
    wfin[:, 0, :] = Wo_w[0:64]      # h_in path
    wfin[:, 1, :] = Wo_w[64:128]    # h_self path
    wfin[:, 2, :] = Wo_w[128:192]   # h_out path
    wfin[:, 3, :] = Wself_w
    bfin = np.zeros((D, 2, 1), f16)
    bfin[:, 0, 0] = Wo_b
    bfin[:, 1, 0] = Wself_b

    # ---- attention edge constants (host) ----
    h_s = s_feat @ Ws_w + Ws_b
    a2 = h_s @ aw2
    ef_ss = inp["efeat_ss"].astype(np.float32)
    ef_os = inp["efeat_os"].astype(np.float32)
    c_ss = ef_ss @ (W2ss @ aw1) + (Wss_b @ aw1 + attn_b[0]) \
        + a2[np.asarray(inp["ss_dst"]).astype(np.int64)]
    c_os = ef_os @ (W2os @ aw1) + (Wos_b @ aw1 + attn_b[0]) \
        + a2[np.asarray(inp["os_dst"]).astype(np.int64)]

    edge_cfg = {
        "ss": (inp["ss_src"], inp["ss_dst"], c_ss, ef_ss, 10),
        "os": (inp["os_src"], inp["os_dst"], c_os, ef_os, 2),
        "fw": (inp["fwd_src"], inp["fwd_dst"], None, None, 0),
        "bw": (inp["bwd_src"], inp["bwd_dst"], None, None, 0),
    }

    in_maps = [dict() for _ in range(NC)]
    for c in range(NC):
        in_maps[c]["feat"] = feat[c]
        in_maps[c]["wc"] = wc
        in_maps[c]["biasx"] = biasx
        in_maps[c]["w2a_ss"] = w2a_ss
        in_maps[c]["w2a_os"] = w2a_os
        in_maps[c]["wfin"] = wfin
        in_maps[c]["bfin"] = bfin

    for t, (src, dst, c_e, ef, ext) in edge_cfg.items():
        src = np.asarray(src).astype(np.int64)
        dst = np.asarray(dst).astype(np.int64)
        E = src.shape[0]
        core = dst // SHARD
        ldst = dst - core * SHARD
        sg = ldst // SUP
        drel = (ldst - sg * SUP).astype(f16)
        r = (src // SHARD) * NODES + (src - (src // SHARD) * SHARD)
        q = r // QROWS
        lsrc = (r - q * QROWS).astype(np.int16)

        gid = ((core * NSG + sg) * NQ + q)
        NG = NC * NSG * NQ
        order = np.argsort(gid, kind="stable")
        cnt = np.bincount(gid, minlength=NG)
        starts = np.zeros(NG + 1, np.int64)
        np.cumsum(cnt, out=starts[1:])
        rank = np.empty(E, np.int64)
        rank[order] = np.arange(E) - starts[gid[order]]
        if not (rank < SLOTS_W).all():
            # ~1e-17 probability; drop excess edges rather than crash
            keep = rank < SLOTS_W
            src, dst, core, sg, q, lsrc, drel, rank = (
                a[keep] for a in (src, dst, core, sg, q, lsrc, drel, rank))
            if c_e is not None:
                c_e, ef = c_e[keep], ef[keep]
        tok = rank

        idx_a = np.zeros((NC, NSG, NQ, TOK), np.int16)
        dr_a = np.full((NC, NSG, NQ, TOK), -1.0, f16)
        idx_a[core, sg, q, tok] = lsrc
        dr_a[core, sg, q, tok] = drel
        # device layouts
        idx_w = np.ascontiguousarray(
            idx_a.reshape(NC, NSG, NQ, TOK // 16, 16).transpose(0, 1, 4, 2, 3))
        # [NC, NSG, 16, NQ, TOK//16]
        dr_w = np.ascontiguousarray(
            dr_a.reshape(NC, NSG, NQ, CHUNKS_SG, 128).transpose(0, 1, 4, 2, 3))
        # [NC, NSG, 128, NQ, CHUNKS_SG]
        for c in range(NC):
            in_maps[c][f"idx_{t}"] = idx_w[c]
            in_maps[c][f"dr_{t}"] = dr_w[c]
        if c_e is not None:
            cc_a = np.zeros((NC, NSG, NQ, TOK), f16)
            cc_a[core, sg, q, tok] = c_e.astype(f16)
            cc_w = np.ascontiguousarray(
                cc_a.reshape(NC, NSG, NQ, CHUNKS_SG, 128).transpose(0, 1, 4, 2, 3))
            ef_a = np.zeros((NC, NSG, NQ, TOK, ext), np.int8)
            ef_a[core, sg, q, tok] = np.clip(
                np.rint(ef / S_EF), -127, 127).astype(np.int8)
            ef_w = np.ascontiguousarray(
                ef_a.reshape(NC, NSG, NQ, CHUNKS_SG, 128, ext)
                .transpose(0, 1, 4, 2, 3, 5))
            for c in range(NC):
                in_maps[c][f"c_{t}"] = cc_w[c]
                in_maps[c][f"ef_{t}"] = ef_w[c]
    return in_maps


# ---------------------------------------------------------------- bass build

def _build_program():
    from concourse import bass, bacc, mybir
    import concourse.tile as tile

    F16 = mybir.dt.float16
    F32 = mybir.dt.float32
    I16 = mybir.dt.int16
    AF = mybir.ActivationFunctionType
    OP = mybir.AluOpType

    nc = bacc.Bacc(None, target_bir_lowering=False, num_devices=NC,
                   dynamic_dma_scratch_size=2 ** 15)

    inp = {}
    inp["feat"] = nc.declare_dram_parameter("feat", [2, D, NODES], F16, isOutput=False)
    inp["wc"] = nc.declare_dram_parameter("wc", [D, 3, 128], F16, isOutput=False)
    inp["biasx"] = nc.declare_dram_parameter("biasx", [128, 1], F32, isOutput=False)
    inp["w2a_ss"] = nc.declare_dram_parameter("w2a_ss", [75, 64], F16, isOutput=False)
    inp["w2a_os"] = nc.declare_dram_parameter("w2a_os", [67, 64], F16, isOutput=False)
    inp["wfin"] = nc.declare_dram_parameter("wfin", [D, 4, 64], F16, isOutput=False)
    inp["bfin"] = nc.declare_dram_parameter("bfin", [D, 2, 1], F32, isOutput=False)
    inp["iden"] = nc.declare_dram_parameter("iden", [128, 128], F16, isOutput=False)
    inp["iota"] = nc.declare_dram_parameter("iota", [128, 1, 128], F16, isOutput=False)
    for t, ext, attn, _, _, _ in TYPES:
        inp[f"idx_{t}"] = nc.declare_dram_parameter(
            f"idx_{t}", [NSG, 16, NQ, TOK // 16], I16, isOutput=False)
        inp[f"dr_{t}"] = nc.declare_dram_parameter(
            f"dr_{t}", [NSG, 128, NQ, CHUNKS_SG], F16, isOutput=False)
        if attn:
            inp[f"c_{t}"] = nc.declare_dram_parameter(
                f"c_{t}", [NSG, 128, NQ, CHUNKS_SG], F16, isOutput=False)
            inp[f"ef_{t}"] = nc.declare_dram_parameter(
                f"ef_{t}", [NSG, 128, NQ, CHUNKS_SG, ext], F16, isOutput=False)
    out = nc.declare_dram_parameter("out", [2, NODES, D], F16, isOutput=True)

    with tile.TileContext(nc) as tc:
        with tc.tile_pool(name="dram", bufs=1, space="DRAM") as dram:
            bounce = dram.tile([2, D, NODES], F16)
            featg = dram.tile([NC, 2, D, NODES], F16, addr_space="Shared")
            tbl = dram.tile([3, TROWS, 128], F16)

            nc.sync.dma_start(out=bounce[:, :, :], in_=inp["feat"][:, :, :])
            nc.gpsimd.collective_compute(
                "AllGather", OP.bypass,
                replica_groups=[list(range(NC))],
                ins=[bounce[:, :, :].opt()],
                outs=[featg[:, :, :, :].opt()],
            )

            # ---------------- phase A: build tables ----------------
            with (
                tc.tile_pool(name="tconst", bufs=1) as tcp,
                tc.tile_pool(name="tbuild", bufs=3) as tp,
                tc.tile_pool(name="tpsum", bufs=2, space="PSUM") as tpp,
            ):
                wc_sb = tcp.tile([D, 3, 128], F16)
                nc.sync.dma_start(out=wc_sb[:, :, :], in_=inp["wc"][:, :, :])
                bx_sb = tcp.tile([128, 1], F16)
                nc.sync.dma_start(out=bx_sb[:, :], in_=inp["biasx"][:, :])
                idA_sb = tcp.tile([128, 128], F16)
                nc.sync.dma_start(out=idA_sb[:, :], in_=inp["iden"][:, :])

                for c8 in range(NC):
                    for t in range(3):
                        srcf = 0 if t == 0 else 1
                        for j0 in range(0, NODES, 512):
                            W = min(512, NODES - j0)
                            KT = W // 128
                            rsb = tp.tile([D, 512], F16, tag="rsb")
                            nc.sync.dma_start(
                                out=rsb[:, :W], in_=featg[c8, srcf, :, j0:j0 + W])
                            ps = tpp.tile([128, 512], F32, tag="psA")
                            nc.tensor.matmul(ps[:, :W], wc_sb[:, t, :], rsb[:, :W],
                                             start=True, stop=True)
                            csb = tp.tile([128, 512], F16, tag="csb")
                            if t == 2:
                                nc.scalar.activation(csb[:, :W], ps[:, :W],
                                                     AF.Identity, bias=bx_sb[:, :])
                            else:
                                nc.scalar.activation(csb[:, :W], ps[:, :W], AF.Copy)
                            ps2 = tpp.tile([128, 4, 128], F32, tag="psA2")
                            for k in range(KT):
                                nc.tensor.matmul(
                                    ps2[:, k, :], csb[:, k * 128:(k + 1) * 128],
                                    idA_sb[:, :], start=True, stop=True)
                            osb = tp.tile([128, 4, 128], F16, tag="osb")
                            if (j0 // 512) % 2 == 0:
                                nc.vector.tensor_copy(out=osb[:, :KT, :],
                                                      in_=ps2[:, :KT, :])
                            else:
                                nc.scalar.activation(osb[:, :KT, :], ps2[:, :KT, :],
                                                     AF.Copy)
                            base = c8 * NODES + j0
                            nc.sync.dma_start(
                                out=tbl[t, base:base + W, :].rearrange(
                                    "(k p) f -> p k f", p=128),
                                in_=osb[:, :KT, :])

            # ---------------- phase B: edges + finalize ----------------
            with (
                tc.tile_pool(name="const", bufs=1) as cp,
                tc.tile_pool(name="acc", bufs=1) as ap_,
                tc.tile_pool(name="work", bufs=2) as wp,
                tc.tile_pool(name="small", bufs=3) as sp,
                tc.tile_pool(name="eps", bufs=3, space="PSUM") as epp,
                tc.tile_pool(name="fin", bufs=1, space="PSUM") as fpp,
            ):
                iden_sb = cp.tile([128, 128], F16)
                nc.sync.dma_start(out=iden_sb[:, :], in_=inp["iden"][:, :])
                iota_sb = cp.tile([128, 1, 128], F16)
                nc.sync.dma_start(out=iota_sb[:, :, :], in_=inp["iota"][:, :, :])
                w2ss_sb = cp.tile([75, 64], F16)
                nc.sync.dma_start(out=w2ss_sb[:, :], in_=inp["w2a_ss"][:, :])
                w2os_sb = cp.tile([67, 64], F16)
                nc.sync.dma_start(out=w2os_sb[:, :], in_=inp["w2a_os"][:, :])
                wfin_sb = cp.tile([D, 4, 64], F16)
                nc.sync.dma_start(out=wfin_sb[:, :, :], in_=inp["wfin"][:, :, :])
                bfin_sb = cp.tile([D, 2, 1], F16)
                nc.sync.dma_start(out=bfin_sb[:, :, :], in_=inp["bfin"][:, :, :])

                for half in range(2):
                    accs = {}
                    for (tname, ext, attn, tq, col0, M) in TYPES:
                        acc = ap_.tile([75, HALF, 128], F16, tag=f"acc_{tname}")
                        accs[tname] = acc
                        for sgl in range(NSG // 2):
                            sg = half * (NSG // 2) + sgl
                            idx_sb = wp.tile([128, NQ, TOK // 16], I16, tag="idx")
                            for k in range(8):
                                nc.sync.dma_start(
                                    out=idx_sb[16 * k:16 * (k + 1), :, :],
                                    in_=inp[f"idx_{tname}"][sg, :, :, :])
                            dr_sb = wp.tile([128, NQ, CHUNKS_SG], F16, tag="dr")
                            nc.sync.dma_start(out=dr_sb[:, :, :],
                                              in_=inp[f"dr_{tname}"][sg, :, :, :])
                            land = wp.tile([128, NQ, CHUNKS_SG, 128], F16, tag="land")
                            for q in range(NQ):
                                nc.gpsimd.dma_gather(
                                    out_ap=land[:, q, :, :],
                                    in_ap=tbl[tq, q * QROWS:(q + 1) * QROWS, :],
                                    idxs_ap=idx_sb[:, q, :],
                                    num_idxs=TOK,
                                    num_idxs_reg=TOK,
                                    elem_size=128,
                                    single_packet=False,
                                )
                            if attn:
                                c_sb = wp.tile([128, NQ, CHUNKS_SG], F16, tag="cc")
                                nc.sync.dma_start(out=c_sb[:, :, :],
                                                  in_=inp[f"c_{tname}"][sg, :, :, :])
                                ef8_sb = wp.tile([128, NQ, CHUNKS_SG, 10],
                                                 mybir.dt.int8, tag="ef8")
                                nc.sync.dma_start(
                                    out=ef8_sb[:, :, :, :ext],
                                    in_=inp[f"ef_{tname}"][sg, :, :, :, :])
                                ef_sb = wp.tile([128, NQ, CHUNKS_SG, 10], F16,
                                                tag="ef")
                                nc.vector.tensor_copy(
                                    out=ef_sb[:, :, :, :ext],
                                    in_=ef8_sb[:, :, :, :ext])
                                sv = wp.tile([128, NQ, CHUNKS_SG, 1], F32, tag="sv")
                                nc.vector.tensor_tensor(
                                    out=sv[:, :, :, :], in0=land[:, :, :, 64:65],
                                    in1=c_sb[:, :, :].unsqueeze(3), op=OP.add)
                                nc.scalar.activation(sv[:, :, :, :], sv[:, :, :, :],
                                                     AF.Lrelu, alpha=0.01)
                                nom = wp.tile([128, NQ, CHUNKS_SG, 1], F16, tag="nom")
                                nc.scalar.activation(nom[:, :, :, :], sv[:, :, :, :],
                                                     AF.Exp)
                                nomS = wp.tile([128, NQ, CHUNKS_SG, 1], F16,
                                               tag="nomS")
                                nc.vector.tensor_scalar_mul(
                                    nomS[:, :, :, :], nom[:, :, :, :], S_EF)
                                U = wp.tile([128, NQ, CHUNKS_SG, 75], F16, tag="U")
                                nc.vector.tensor_tensor(
                                    out=U[:, :, :, 0:64], in0=land[:, :, :, 0:64],
                                    in1=nom[:, :, :, :].to_broadcast(
                                        [128, NQ, CHUNKS_SG, 64]),
                                    op=OP.mult)
                                nc.vector.tensor_tensor(
                                    out=U[:, :, :, 65:65 + ext],
                                    in0=ef_sb[:, :, :, :ext],
                                    in1=nomS[:, :, :, :].to_broadcast(
                                        [128, NQ, CHUNKS_SG, ext]),
                                    op=OP.mult)
                                nc.scalar.activation(
                                    U[:, :, :, 64:65], nom[:, :, :, :], AF.Copy)
                            for wl in range(SG):
                                S = sp.tile([128, NQ, B, 128], F16, tag="S")
                                nc.vector.tensor_tensor(
                                    out=S[:, :, :, :],
                                    in0=dr_sb[:, :, wl * B:(wl + 1) * B]
                                    .unsqueeze(3).to_broadcast([128, NQ, B, 128]),
                                    in1=iota_sb[:, 0:1, :].unsqueeze(1)
                                    .to_broadcast([128, NQ, B, 128]),
                                    op=OP.is_equal)
                                ps = epp.tile([75, 128], F32, tag="eps")
                                for q in range(NQ):
                                    for j in range(B):
                                        ch = wl * B + j
                                        if attn:
                                            lhsT = U[:, q, ch, 0:M]
                                        else:
                                            lhsT = land[:, q, ch, col0:col0 + 64]
                                        nc.tensor.matmul(
                                            ps[0:M, :], lhsT, S[:, q, j, :],
                                            start=(q == 0 and j == 0),
                                            stop=(q == NQ - 1 and j == B - 1))
                                wloc = sgl * SG + wl
                                nc.vector.tensor_copy(out=acc[0:M, wloc, :],
                                                      in_=ps[0:M, :])
                    # ---- finalize this half ----
                    for wloc in range(HALF):
                        n0 = (half * HALF + wloc) * 128
                        a_ss, a_os = accs["ss"], accs["os"]
                        a_fw, a_bw = accs["fw"], accs["bw"]
                        nm = fpp.tile([64, 2, 128], F32, tag="nm")
                        nc.tensor.matmul(nm[:, 0, :], iden_sb[0:64, 0:64],
                                         a_ss[0:64, wloc, :], start=True, stop=False)
                        nc.tensor.matmul(nm[:, 0, :], w2ss_sb[64:75, :],
                                         a_ss[64:75, wloc, :], start=False, stop=True)
                        nc.tensor.matmul(nm[:, 1, :], iden_sb[0:64, 0:64],
                                         a_os[0:64, wloc, :], start=True, stop=False)
                        nc.tensor.matmul(nm[:, 1, :], w2os_sb[64:67, :],
                                         a_os[64:67, wloc, :], start=False, stop=True)
                        dT = fpp.tile([128, 2, 1], F32, tag="dT")
                        nc.tensor.matmul(dT[:, 0, :], a_ss[64:65, wloc, :],
                                         iden_sb[64:65, 64:65], start=True, stop=True)
                        nc.tensor.matmul(dT[:, 1, :], a_os[64:65, wloc, :],
                                         iden_sb[64:65, 64:65], start=True, stop=True)
                        dmx = sp.tile([128, 2, 1], F32, tag="dmx")
                        nc.vector.tensor_scalar_max(dmx[:, :, :], dT[:, :, :], 1e-20)
                        rec = sp.tile([128, 2, 1], F32, tag="rec")
                        nc.vector.reciprocal(rec[:, :, :], dmx[:, :, :])
                        nsb = sp.tile([64, 2, 128], F16, tag="nsb")
                        nc.scalar.activation(nsb[:, :, :], nm[:, :, :], AF.Copy)
                        zT = fpp.tile([128, 2, 64], F32, tag="zT")
                        nc.tensor.matmul(zT[:, 0, :], nsb[:, 0, :],
                                         iden_sb[0:64, 0:64], start=True, stop=True)
                        nc.tensor.matmul(zT[:, 1, :], nsb[:, 1, :],
                                         iden_sb[0:64, 0:64], start=True, stop=True)
                        zp = sp.tile([128, 2, 64], F32, tag="zp")
                        nc.vector.tensor_scalar(
                            out=zp[:, 0, :], in0=zT[:, 0, :],
                            scalar1=rec[:, 0, :], scalar2=None, op0=OP.mult)
                        nc.vector.tensor_scalar(
                            out=zp[:, 1, :], in0=zT[:, 1, :],
                            scalar1=rec[:, 1, :], scalar2=None, op0=OP.mult)
                        zo = sp.tile([128, 64], F16, tag="zo")
                        nc.vector.tensor_tensor(out=zo[:, :], in0=zp[:, 0, :],
                                                in1=zp[:, 1, :], op=OP.add)
                        nc.sync.dma_start(out=out[0, n0:n0 + 128, :], in_=zo[:, :])
                        # x path
                        fsb = sp.tile([64, 128], F16, tag="fsb")
                        nc.sync.dma_start(out=fsb[:, :],
                                          in_=inp["feat"][1, :, n0:n0 + 128])
                        hx = fpp.tile([64, 2, 128], F32, tag="hx")
                        nc.tensor.matmul(hx[:, 0, :], wfin_sb[:, 3, :], fsb[:, :],
                                         start=True, stop=True)
                        rl = sp.tile([64, 3, 128], F16, tag="rl")
                        nc.scalar.activation(rl[:, 0, :], a_fw[0:64, wloc, :],
                                             AF.Relu)
                        nc.scalar.activation(rl[:, 1, :], hx[:, 0, :], AF.Relu,
                                             bias=bfin_sb[:, 1, :])
                        nc.scalar.activation(rl[:, 2, :], a_bw[0:64, wloc, :],
                                             AF.Relu)
                        nc.tensor.matmul(hx[:, 1, :], wfin_sb[:, 0, :], rl[:, 0, :],
                                         start=True, stop=False)
                        nc.tensor.matmul(hx[:, 1, :], wfin_sb[:, 1, :], rl[:, 1, :],
                                         start=False, stop=False)
                        nc.tensor.matmul(hx[:, 1, :], wfin_sb[:, 2, :], rl[:, 2, :],
                                         start=False, stop=True)
                        xsb = sp.tile([64, 128], F16, tag="xsb")
                        nc.scalar.activation(xsb[:, :], hx[:, 1, :], AF.Identity,
                                             bias=bfin_sb[:, 0, :])
                        xT = fpp.tile([128, 64], F32, tag="xT")
                        nc.tensor.matmul(xT[:, :], xsb[:, :], iden_sb[0:64, 0:64],
                                         start=True, stop=True)
                        xo = sp.tile([128, 64], F16, tag="xo")
                        nc.scalar.activation(xo[:, :], xT[:, :], AF.Copy)
                        nc.sync.dma_start(out=out[1, n0:n0 + 128, :], in_=xo[:, :])

    nc.finalize()
    return nc


# ---------------------------------------------------------------- numpy sim

def _simulate(in_maps):
    """Numpy emulation of the device program (fp16 rounding where it
    matters) — validates packing + math without compiling."""
    f16 = np.float16
    results = []
    for c in range(NC):
        results.append({})
    # AllGather
    featg = np.stack([in_maps[c]["feat"] for c in range(NC)])  # [NC,2,64,NODES]
    wc = in_maps[0]["wc"].astype(np.float32)
    biasx = in_maps[0]["biasx"].astype(np.float32)
    # tables (same on all cores)
    tbl = np.zeros((3, TROWS, 128), f16)
    for t in range(3):
        srcf = 0 if t == 0 else 1
        ft = featg[:, srcf].transpose(0, 2, 1).reshape(TROWS, D)  # [TROWS, 64]
        m = ft.astype(np.float32) @ wc[:, t, :]
        if t == 2:
            m = m + biasx[:, 0][None, :]
        tbl[t] = m.astype(f16)

    for c in range(NC):
        im = in_maps[c]
        out = np.zeros((2, NODES, D), f16)
        acc_all = {}
        for (tname, ext, attn, tq, col0, M) in TYPES:
            acc = np.zeros((M, NWIN, 128), f16)
            for sg in range(NSG):
                idx = im[f"idx_{tname}"][sg]      # [16, NQ, 224]
                dr = im[f"dr_{tname}"][sg]        # [128, NQ, 28]
                # reconstruct tokens: token t at [t%16, q, t//16]
                toks = idx.transpose(1, 2, 0).reshape(NQ, TOK)  # [NQ, TOK]
                land = np.zeros((128, NQ, CHUNKS_SG, 128), f16)
                for q in range(NQ):
                    g = tbl[tq, q * QROWS + toks[q].astype(np.int64), :]
                    land[:, q, :, :] = g.reshape(CHUNKS_SG, 128, 128).transpose(1, 0, 2)
                if attn:
                    cc = im[f"c_{tname}"][sg]     # [128, NQ, 28]
                    ef = im[f"ef_{tname}"][sg]    # [128, NQ, 28, ext]
                    sv = land[:, :, :, 64].astype(np.float32) + cc.astype(np.float32)
                    sv = np.where(sv > 0, sv, 0.01 * sv)
                    nom = np.exp(sv).astype(f16)
                    U = np.zeros((128, NQ, CHUNKS_SG, M), f16)
                    U[..., 0:64] = (land[..., 0:64].astype(np.float32)
                                    * nom.astype(np.float32)[..., None]).astype(f16)
                    nomS = (nom.astype(np.float32) * S_EF).astype(f16)
                    U[..., 65:65 + ext] = (ef.astype(np.float32)
                                           * nomS.astype(np.float32)[..., None]).astype(f16)
                    U[..., 64] = nom
                for wl in range(SG):
                    S = (dr[:, :, wl * B:(wl + 1) * B, None]
                         == np.arange(128, dtype=f16)[None, None, None, :])
                    ps = np.zeros((M, 128), np.float32)
                    for q in range(NQ):
                        for j in range(B):
                            ch = wl * B + j
                            if attn:
                                lhsT = U[:, q, ch, :].astype(np.float32)
                            else:
                                lhsT = land[:, q, ch, col0:col0 + 64].astype(np.float32)
                            ps += lhsT.T @ S[:, q, j, :].astype(np.float32)
                    acc[:, sg * SG + wl, :] = ps.astype(f16)
            acc_all[tname] = acc
        # finalize
        w2ss = im["w2a_ss"][64:75].astype(np.float32)
        w2os = im["w2a_os"][64:67].astype(np.float32)
        wfin = im["wfin"].astype(np.float32)
        bfin = im["bfin"].astype(np.float32)
        for w in range(NWIN):
            n0 = w * 128
            a_ss = acc_all["ss"][:, w, :].astype(np.float32)
            a_os = acc_all["os"][:, w, :].astype(np.float32)
            num_ss = a_ss[0:64] + w2ss.T @ a_ss[64:75]
            num_os = a_os[0:64] + w2os.T @ a_os[64:67]
            den_ss = np.maximum(a_ss[64], 1e-20)
            den_os = np.maximum(a_os[64], 1e-20)
            z = (num_ss.astype(f16).astype(np.float32) / den_ss[None, :]
                 + num_os.astype(f16).astype(np.float32) / den_os[None, :])
            out[0, n0:n0 + 128, :] = z.T.astype(f16)
            fsb = im["feat"][1, :, n0:n0 + 128].astype(np.float32)
            hself = wfin[:, 3, :].T @ fsb + bfin[:, 1, :]
            r_fw = np.maximum(acc_all["fw"][0:64, w, :].astype(np.float32), 0)
            r_self = np.maximum(hself, 0).astype(f16).astype(np.float32)
            r_bw = np.maximum(acc_all["bw"][0:64, w, :].astype(np.float32), 0)
            x = (wfin[:, 0, :].T @ r_fw.astype(f16).astype(np.float32)
                 + wfin[:, 1, :].T @ r_self
                 + wfin[:, 2, :].T @ r_bw.astype(f16).astype(np.float32)
                 + bfin[:, 0, :])
            out[1, n0:n0 + 128, :] = x.T.astype(f16)
        results[c]["out"] = out
    return results


def _assemble(results):
    z = np.concatenate(
        [results[c]["out"][0, :SHARD, :].astype(np.float32) for c in range(NC)],
        axis=0)
    x = np.concatenate(
        [results[c]["out"][1, :SHARD, :].astype(np.float32) for c in range(NC)],
        axis=0)
    return z, x


def kernel_sim(**inputs):
    inp = {k: np.asarray(v) for k, v in inputs.items()}
    in_maps = _pack(inp)
    return _assemble(_simulate(in_maps))


def _blobify(in_maps):
    offs, total = _blob_spec()
    blobs = np.empty((NC, total), np.int16)
    for c in range(NC):
        b = blobs[c]
        for name, (off, shape, dt) in offs.items():
            a = in_maps[c][name].ravel()
            v = a.view(np.int16)
            b[off:off + v.shape[0]] = v
    return blobs


def _get_runner():
    global _PROGRAM, _RUNNER
    if _RUNNER is not None:
        return _RUNNER
    import jax, jax.numpy as jnp
    from jax.sharding import Mesh, PartitionSpec, NamedSharding
    from jax.experimental.shard_map import shard_map
    from concourse import mybir
    from concourse.bass2jax import (_bass_exec_p, install_neuronx_cc_hook,
                                    partition_id_tensor)
    if _PROGRAM is None:
        _PROGRAM = _build_program()
    nc_ = _PROGRAM
    install_neuronx_cc_hook()
    partition_name = (nc_.partition_id_tensor.name
                      if nc_.partition_id_tensor else None)
    in_names, out_names, out_avals, zero_specs = [], [], [], []
    for alloc in nc_.m.functions[0].allocations:
        if not isinstance(alloc, mybir.MemoryLocationSet):
            continue
        if alloc.kind not in ("ExternalInput", "ExternalOutput"):
            continue
        name = alloc.memorylocations[0].name
        if alloc.kind == "ExternalInput":
            if name != partition_name:
                in_names.append(name)
        else:
            shape = tuple(alloc.tensor_shape)
            dtype = mybir.dt.np(alloc.dtype)
            out_names.append(name)
            out_avals.append(jax.core.ShapedArray(shape, dtype))
            zero_specs.append((shape, dtype))
    n_params = len(in_names)
    bind_names = tuple(in_names + out_names
                       + ([partition_name] if partition_name else []))
    donate = tuple(range(n_params, n_params + len(out_names)))

    def _body(*args):
        operands = list(args)
        if partition_name is not None:
            operands.append(partition_id_tensor())
        outs = _bass_exec_p.bind(
            *operands, out_avals=tuple(out_avals), in_names=bind_names,
            out_names=tuple(out_names), lowering_input_output_aliases=(),
            sim_require_finite=True, sim_require_nnan=True, nc=nc_)
        return tuple(outs)

    devs = jax.devices()[:NC]
    mesh = Mesh(np.asarray(devs), ("core",))
    ns = NamedSharding(mesh, PartitionSpec("core"))
    nin = n_params + len(out_names)
    sharded = jax.jit(
        shard_map(_body, mesh=mesh, in_specs=(PartitionSpec("core"),) * nin,
                  out_specs=(PartitionSpec("core"),) * len(out_names),
                  check_rep=False),
        donate_argnums=donate, keep_unused=True)
    zeros_fn = jax.jit(
        lambda: tuple(jnp.zeros((NC * s[0], *s[1:]), d) for s, d in zero_specs),
        out_shardings=(ns,) * len(zero_specs))
    _RUNNER = (sharded, zeros_fn, in_names, out_names, mesh, devs, ns)
    return _RUNNER


def kernel(**inputs):
    global LAST_DEVICE_WALL_NS
    import time as _time
    import jax
    from concurrent.futures import ThreadPoolExecutor
    inp = {k: np.asarray(v) for k, v in inputs.items()}
    in_maps = _pack(inp)
    blobs = _blobify(in_maps)
    sharded, zeros_fn, in_names, out_names, mesh, devs, ns = _get_runner()
    assert in_names == ["blob"], in_names
    _t0 = _time.time()

    def put_one(c):
        a = jax.device_put(blobs[c], devs[c])
        a.block_until_ready()
        return a

    with ThreadPoolExecutor(NC) as ex:
        bufs = list(ex.map(put_one, range(NC)))
    garr = jax.make_array_from_single_device_arrays(
        (NC * blobs.shape[1],), ns, bufs)
    zeros = zeros_fn()
    outs = sharded(garr, *zeros)
    jax.block_until_ready(outs)
    shards = sorted(outs[0].addressable_shards,
                    key=lambda sh: (sh.index[0].start or 0))
    with ThreadPoolExecutor(NC) as ex:
        parts = list(ex.map(lambda sh: np.asarray(sh.data), shards))
    LAST_DEVICE_WALL_NS = (_time.time() - _t0) * 1e9
    results = [{"out": parts[c]} for c in range(NC)]
    return _assemble(results)
